# revision 1
# baseline (speedup 1.0000x reference)
"""Trainium2 Bass kernel for nn_CysInteractLayer (GNN message-passing layer).

out = BN(lrelu(lrelu(lrelu([ef | nf[src]+nf[dst]] @ W1 + b1) @ W2 + b2) @ W3 + b3))

Sharding: edges across 8 NeuronCores; node_feats/weights replicated
(as per-half local node tables so dma_gather's int16 indices suffice);
BN batch stats all-reduced across cores on-device.
"""
import numpy as np
import ml_dtypes

import concourse.bass as bass
import concourse.bacc as bacc
import concourse.tile as tile
from concourse import mybir
from concourse.bass_utils import run_bass_kernel_spmd
from concourse.masks import make_identity

F32 = mybir.dt.float32
BF16 = mybir.dt.bfloat16
I16 = mybir.dt.int16

# problem constants (hardcoded per harness contract)
V, E = 50000, 400000
ND, ED, OUT = 128, 64, 128
IN = ND + ED
NEG_SLOPE = 0.01
BN_EPS = 1e-5

NCORES = 8
ES = E // NCORES            # 50000 edges per core


class Cfg:
    """Geometry of the per-core kernel; small instances used for sim tests."""

    def __init__(self, es=ES, halves=2, calls_per_half=7, blocks_per_call=7,
                 tn=32768, e_total=E, use_ttr=False, per_tile_groups=False,
                 skip_collective=False, skip_gather=False, gq=1):
        self.use_ttr = use_ttr
        self.per_tile_groups = per_tile_groups
        self.skip_collective = skip_collective
        self.skip_gather = skip_gather
        self.gq = gq
        self.ES = es                        # valid edges per core
        self.H = halves
        self.C = calls_per_half
        self.B = blocks_per_call
        self.BLK = 512
        self.TN = tn                        # local table rows (padded)
        self.E_TOTAL = e_total
        self.HP = self.C * self.B * self.BLK   # padded edges per half
        self.EP = self.H * self.HP             # padded edges per core
        self.HV = es // halves                 # valid edges per half
        assert es % halves == 0
        assert self.HP >= self.HV
        self.NB = self.H * self.C * self.B     # total blocks
        self.NCALL = self.B * self.BLK         # idxs per gather call
        self.WCOLS = self.NCALL // 16          # wrapped idx cols per call

    def block_valid(self, b):
        """Valid columns in global block b (rest is padding)."""
        h, lb = divmod(b, self.C * self.B)
        lo = lb * self.BLK
        return int(np.clip(self.HV - lo, 0, self.BLK))


CFG = Cfg()

_PROG_CACHE = {}


def build_program(cfg):
    nc = bacc.Bacc(None, target_bir_lowering=False, num_swdge_queues=4)

    tabs = [nc.declare_dram_parameter(f"tab{h}", [cfg.TN, ND], BF16, isOutput=False)
            for h in range(cfg.H)]
    srcw = nc.declare_dram_parameter("srcw", [128, 2 * cfg.H * cfg.C * cfg.WCOLS], I16, isOutput=False)
    dstw = nc.declare_dram_parameter("dstw", [128, 2 * cfg.H * cfg.C * cfg.WCOLS], I16, isOutput=False)
    eft = nc.declare_dram_parameter("eft", [ED, cfg.EP], BF16, isOutput=False)
    w1e = nc.declare_dram_parameter("w1e", [ED, OUT], BF16, isOutput=False)
    w1m = nc.declare_dram_parameter("w1m", [ND, OUT], BF16, isOutput=False)
    w2 = nc.declare_dram_parameter("w2", [OUT, OUT], BF16, isOutput=False)
    w3 = nc.declare_dram_parameter("w3", [OUT, OUT], BF16, isOutput=False)
    bias = nc.declare_dram_parameter("bias", [128, 3], F32, isOutput=False)   # b1|b2|b3
    gb = nc.declare_dram_parameter("gb", [128, 2], F32, isOutput=False)       # gamma|beta
    out = nc.declare_dram_parameter("out", [cfg.EP, OUT], F32, isOutput=True)

    LR = mybir.ActivationFunctionType.Lrelu
    IDENT = mybir.ActivationFunctionType.Identity

    with tile.TileContext(nc) as tc:
        with (
            tc.tile_pool(name="singles", bufs=1) as singles,
            tc.tile_pool(name="hres", bufs=1) as hres,
            tc.tile_pool(name="gat", bufs=2) as gat,
            tc.tile_pool(name="work", bufs=3) as work,
            tc.tile_pool(name="zps", bufs=4, space="PSUM") as zps,
            tc.tile_pool(name="ops", bufs=3, space="PSUM") as ops,
            tc.tile_pool(name="dram", bufs=1, space="DRAM") as dram,
        ):
            # ---- static state ------------------------------------------------
            w1e_t = singles.tile([ED, OUT], BF16)
            nc.sync.dma_start(out=w1e_t[:], in_=w1e[:, :])
            w1m_t = singles.tile([ND, OUT], BF16)
            nc.sync.dma_start(out=w1m_t[:], in_=w1m[:, :])
            w2_t = singles.tile([OUT, OUT], BF16)
            nc.sync.dma_start(out=w2_t[:], in_=w2[:, :])
            w3_t = singles.tile([OUT, OUT], BF16)
            nc.sync.dma_start(out=w3_t[:], in_=w3[:, :])
            bias_t = singles.tile([128, 3], F32)
            nc.sync.dma_start(out=bias_t[:], in_=bias[:, :])
            gb_t = singles.tile([128, 2], F32)
            nc.sync.dma_start(out=gb_t[:], in_=gb[:, :])
            srcw_t = singles.tile([128, 2 * cfg.H * cfg.C * cfg.WCOLS], I16)
            nc.sync.dma_start(out=srcw_t[:], in_=srcw[:, :])
            dstw_t = singles.tile([128, 2 * cfg.H * cfg.C * cfg.WCOLS], I16)
            nc.sync.dma_start(out=dstw_t[:], in_=dstw[:, :])
            ident_f32 = singles.tile([128, 128], F32)
            make_identity(nc, ident_f32[:])

            h3res = hres.tile([128, cfg.EP], BF16)
            sum_stripe = singles.tile([128, cfg.NB], F32)
            sq_stripe = singles.tile([128, cfg.NB], F32)

            # ---- phase 1: gather + MLP + stats -------------------------------
            for h in range(cfg.H):
                for c in range(cfg.C):
                    wofs = (h * cfg.C + c) * cfg.WCOLS
                    gsd = gat.tile([128, 1, 2 * cfg.NCALL], BF16, tag="gsd")
                    if cfg.skip_gather:
                        nc.vector.memset(gsd[:], 0.25)
                    else:
                        nc.gpsimd.dma_gather(
                            out_ap=gsd[:], in_ap=tabs[h][:, :],
                            idxs_ap=srcw_t[:, 2 * wofs:2 * (wofs + cfg.WCOLS)],
                            num_idxs=2 * cfg.NCALL, num_idxs_reg=2 * cfg.NCALL,
                            elem_size=ND, transpose=True, single_packet=False,
                        )
                    gs = gsd[:, :, :cfg.NCALL]
                    gd = gsd[:, :, cfg.NCALL:]
                    call_e0 = (h * cfg.C + c) * cfg.NCALL
                    eft_t = gat.tile([ED, cfg.NCALL], BF16, tag="eft")
                    nc.sync.dma_start(out=eft_t[:], in_=eft[:, call_e0:call_e0 + cfg.NCALL])

                    for b in range(cfg.B):
                        gb_idx = (h * cfg.C + c) * cfg.B + b
                        vb = cfg.block_valid(gb_idx)
                        if vb == 0:
                            continue
                        co = b * cfg.BLK          # col offset within call
                        e0 = call_e0 + co         # global (padded) edge offset
                        zp = zps.tile([128, cfg.BLK], F32, tag="z")
                        nc.tensor.matmul(zp[:, :], lhsT=w1m_t[:], rhs=gs[:, 0, co:co + cfg.BLK],
                                         start=True, stop=False)
                        nc.tensor.matmul(zp[:, :], lhsT=w1m_t[:], rhs=gd[:, 0, co:co + cfg.BLK],
                                         start=False, stop=False)
                        nc.tensor.matmul(zp[:, :], lhsT=w1e_t[:], rhs=eft_t[:, co:co + cfg.BLK],
                                         start=False, stop=True)
                        h1 = work.tile([128, cfg.BLK], BF16, tag="h1")
                        nc.scalar.activation(out=h1[:], in_=zp[:], func=LR,
                                             bias=bias_t[:, 0:1], scale=1.0, alpha=NEG_SLOPE)
                        zp2 = zps.tile([128, cfg.BLK], F32, tag="z")
                        nc.tensor.matmul(zp2[:, :], lhsT=w2_t[:], rhs=h1[:], start=True, stop=True)
                        h2 = work.tile([128, cfg.BLK], BF16, tag="h2")
                        nc.scalar.activation(out=h2[:], in_=zp2[:], func=LR,
                                             bias=bias_t[:, 1:2], scale=1.0, alpha=NEG_SLOPE)
                        zp3 = zps.tile([128, cfg.BLK], F32, tag="z")
                        nc.tensor.matmul(zp3[:, :], lhsT=w3_t[:], rhs=h2[:], start=True, stop=True)
                        h3 = h3res[:, e0:e0 + cfg.BLK]
                        nc.scalar.activation(out=h3[:, :vb], in_=zp3[:, :vb], func=LR,
                                             bias=bias_t[:, 2:3], scale=1.0, alpha=NEG_SLOPE,
                                             accum_out=sum_stripe[:, gb_idx:gb_idx + 1])
                        if vb < cfg.BLK:
                            nc.vector.memset(h3[:, vb:], 0.0)
                        sq_scr = work.tile([128, cfg.BLK], BF16, tag="sq")
                        nc.vector.tensor_mul(out=sq_scr[:], in0=h3[:, :], in1=h3[:, :])
                        nc.vector.tensor_reduce(
                            out=sq_stripe[:, gb_idx:gb_idx + 1], in_=sq_scr[:],
                            axis=mybir.AxisListType.X, op=mybir.AluOpType.add)

            # ---- stats + allreduce ------------------------------------------
            st2 = singles.tile([128, 2], F32)
            nc.vector.tensor_reduce(out=st2[:, 0:1], in_=sum_stripe[:],
                                    axis=mybir.AxisListType.X, op=mybir.AluOpType.add)
            nc.vector.tensor_reduce(out=st2[:, 1:2], in_=sq_stripe[:],
                                    axis=mybir.AxisListType.X, op=mybir.AluOpType.add)
            cc_in = dram.tile([128, 2], F32)
            cc_out = dram.tile([128, 2], F32)
            nc.gpsimd.dma_start(out=cc_in[:], in_=st2[:])
            if cfg.skip_collective:
                nc.gpsimd.dma_start(out=cc_out[:], in_=cc_in[:])
            else:
                nc.gpsimd.collective_compute(
                    "AllReduce", mybir.AluOpType.add,
                    replica_groups=[list(range(NCORES))],
                    ins=[cc_in.opt()], outs=[cc_out.opt()],
                )
            gst = singles.tile([128, 2], F32)
            nc.gpsimd.dma_start(out=gst[:], in_=cc_out[:])

            inv_e = 1.0 / cfg.E_TOTAL
            mean_t = singles.tile([128, 1], F32)
            nc.scalar.mul(out=mean_t[:], in_=gst[:, 0:1], mul=inv_e)
            msq_t = singles.tile([128, 1], F32)
            nc.scalar.mul(out=msq_t[:], in_=gst[:, 1:2], mul=inv_e)
            var_t = singles.tile([128, 1], F32)
            nc.vector.tensor_tensor(out=var_t[:], in0=mean_t[:], in1=mean_t[:],
                                    op=mybir.AluOpType.mult)
            nc.vector.tensor_tensor(out=var_t[:], in0=msq_t[:], in1=var_t[:],
                                    op=mybir.AluOpType.subtract)
            eps_t = singles.tile([128, 1], F32)
            nc.vector.memset(eps_t[:], BN_EPS)
            sd_t = singles.tile([128, 1], F32)
            nc.scalar.activation(out=sd_t[:], in_=var_t[:],
                                 func=mybir.ActivationFunctionType.Sqrt,
                                 bias=eps_t[:], scale=1.0)
            rstd_t = singles.tile([128, 1], F32)
            nc.vector.reciprocal(out=rstd_t[:], in_=sd_t[:])
            s_t = singles.tile([128, 1], F32)
            nc.vector.tensor_tensor(out=s_t[:], in0=rstd_t[:], in1=gb_t[:, 0:1],
                                    op=mybir.AluOpType.mult)
            t_t = singles.tile([128, 1], F32)
            nc.vector.tensor_tensor(out=t_t[:], in0=s_t[:], in1=mean_t[:],
                                    op=mybir.AluOpType.mult)
            nc.vector.tensor_tensor(out=t_t[:], in0=gb_t[:, 1:2], in1=t_t[:],
                                    op=mybir.AluOpType.subtract)

            # ---- phase 2: affine + transpose + store ------------------------
            for gb_idx in range(cfg.NB):
                if cfg.block_valid(gb_idx) == 0:
                    continue
                e0 = gb_idx * cfg.BLK
                u = work.tile([128, cfg.BLK], F32, tag="u")
                nc.scalar.activation(out=u[:], in_=h3res[:, e0:e0 + cfg.BLK],
                                     func=IDENT, bias=t_t[:], scale=s_t[:])
                op = ops.tile([128, cfg.BLK], F32, tag="op")
                nsub = cfg.BLK // 128
                for t in range(nsub):
                    st_, sp_ = ((True, True) if cfg.per_tile_groups
                                else (t == 0, t == nsub - 1))
                    nc.tensor.matmul(op[:, t * 128:(t + 1) * 128],
                                     lhsT=u[:, t * 128:(t + 1) * 128], rhs=ident_f32[:],
                                     is_transpose=True, start=st_, stop=sp_)
                ob = work.tile([128, cfg.BLK], F32, tag="ob")
                nc.vector.tensor_copy(out=ob[:], in_=op[:])
                dst_ap = out[e0:e0 + cfg.BLK, :].rearrange("(t p) f -> p t f", p=128)
                nc.sync.dma_start(out=dst_ap, in_=ob[:].rearrange("p (t f) -> p t f", f=128))
    nc.compile()
    return nc


def get_program(cfg):
    key = (cfg.ES, cfg.H, cfg.C, cfg.B, cfg.TN, cfg.E_TOTAL,
           cfg.use_ttr, cfg.per_tile_groups, cfg.skip_collective, cfg.skip_gather, cfg.gq)
    if key not in _PROG_CACHE:
        _PROG_CACHE[key] = build_program(cfg)
    return _PROG_CACHE[key]


def _wrap_idx(flat, cfg):
    """int16 flat idxs [n] -> wrapped [128, n/16] layout (i at [i%16, i//16], x8)."""
    w = flat.reshape(-1, 16).T.astype(np.int16)      # [16, n/16]
    return np.tile(w, (8, 1))                        # [128, n/16]


def host_prep(node_feats, edge_feats, src, dst, W1, b1, W2, b2, W3, b3, gamma, beta,
              cfg=None):
    cfg = cfg or CFG
    nfb = np.asarray(node_feats, np.float32).astype(ml_dtypes.bfloat16)
    efb = np.asarray(edge_feats, np.float32).astype(ml_dtypes.bfloat16)
    src = np.asarray(src)
    dst = np.asarray(dst)
    W1 = np.asarray(W1, np.float32)

    w1e = W1[:ED].astype(ml_dtypes.bfloat16)
    w1m = W1[ED:].astype(ml_dtypes.bfloat16)
    w2b = np.asarray(W2, np.float32).astype(ml_dtypes.bfloat16)
    w3b = np.asarray(W3, np.float32).astype(ml_dtypes.bfloat16)
    bias = np.stack([np.asarray(b1, np.float32),
                     np.asarray(b2, np.float32),
                     np.asarray(b3, np.float32)], axis=1)          # [128, 3]
    gb = np.stack([np.asarray(gamma, np.float32),
                   np.asarray(beta, np.float32)], axis=1)          # [128, 2]

    in_maps = []
    for c in range(NCORES):
        base = c * cfg.ES
        tabs, sws, dws = [], [], []
        for h in range(cfg.H):
            lo = base + h * cfg.HV
            s_h = src[lo:lo + cfg.HV]
            d_h = dst[lo:lo + cfg.HV]
            u = np.unique(np.concatenate([s_h, d_h]))
            assert len(u) <= cfg.TN, f"local table overflow: {len(u)} > {cfg.TN}"
            assert len(u) <= 32768, "int16 index overflow"
            tab = np.zeros((cfg.TN, ND), ml_dtypes.bfloat16)
            tab[:len(u)] = nfb[u]
            tabs.append(tab)
            s16 = np.searchsorted(u, s_h).astype(np.int16)
            d16 = np.searchsorted(u, d_h).astype(np.int16)
            pad = cfg.HP - cfg.HV
            if pad:
                s16 = np.concatenate([s16, np.zeros(pad, np.int16)])
                d16 = np.concatenate([d16, np.zeros(pad, np.int16)])
            # one wrapped array per gather call: src block then dst block merged
            for cl in range(cfg.C):
                sws.append(_wrap_idx(s16[cl * cfg.NCALL:(cl + 1) * cfg.NCALL], cfg))
                sws.append(_wrap_idx(d16[cl * cfg.NCALL:(cl + 1) * cfg.NCALL], cfg))
                dws.append(np.zeros((128, cfg.WCOLS), np.int16))
                dws.append(np.zeros((128, cfg.WCOLS), np.int16))
        srcw = np.concatenate(sws, axis=1)
        dstw = np.concatenate(dws, axis=1)
        # edge feats, transposed + per-half padding
        eftc = np.zeros((ED, cfg.EP), ml_dtypes.bfloat16)
        for h in range(cfg.H):
            lo = base + h * cfg.HV
            eftc[:, h * cfg.HP:h * cfg.HP + cfg.HV] = efb[lo:lo + cfg.HV].T
        im = {"srcw": srcw, "dstw": dstw, "eft": eftc,
              "w1e": w1e, "w1m": w1m, "w2": w2b, "w3": w3b,
              "bias": bias, "gb": gb}
        for h in range(cfg.H):
            im[f"tab{h}"] = tabs[h]
        in_maps.append(im)
    return in_maps


def assemble_output(results, cfg=None):
    cfg = cfg or CFG
    out = np.empty((NCORES * cfg.ES, OUT), np.float32)
    for c in range(NCORES):
        oc = np.asarray(results[c]["out"])
        for h in range(cfg.H):
            lo = c * cfg.ES + h * cfg.HV
            out[lo:lo + cfg.HV] = oc[h * cfg.HP:h * cfg.HP + cfg.HV]
    return out


def kernel(**inputs):
    cfg = CFG
    nc = get_program(cfg)
    in_maps = host_prep(**inputs, cfg=cfg)
    res = run_bass_kernel_spmd(nc, in_maps, list(range(NCORES)))
    return assemble_output(res.results, cfg)



# revision 4
# speedup vs baseline: 1.0474x; 1.0474x over previous
"""Trainium2 Bass kernel for nn_CysInteractLayer (GNN message-passing layer).

out = BN(lrelu(lrelu(lrelu([ef | nf[src]+nf[dst]] @ W1 + b1) @ W2 + b2) @ W3 + b3))

Sharding: edges across 8 NeuronCores; node_feats/weights replicated
(as per-half local node tables so dma_gather's int16 indices suffice);
BN batch stats all-reduced across cores on-device.

v2: feature-major bf16 output (host un-transposes; removes on-chip PE
transposes), bn_stats/bn_aggr for BN statistics, DVE affine in phase 2,
optional multi-queue gathers (gq>1) with per-queue warmup.
"""
import numpy as np
import ml_dtypes

import concourse.bass as bass
import concourse.bacc as bacc
import concourse.tile as tile
from concourse import mybir
from concourse.bass_utils import run_bass_kernel_spmd

F32 = mybir.dt.float32
BF16 = mybir.dt.bfloat16
I16 = mybir.dt.int16

# problem constants (hardcoded per harness contract)
V, E = 50000, 400000
ND, ED, OUT = 128, 64, 128
IN = ND + ED
NEG_SLOPE = 0.01
BN_EPS = 1e-5

NCORES = 8
ES = E // NCORES            # 50000 edges per core


class Cfg:
    """Geometry of the per-core kernel; small instances used for sim tests."""

    def __init__(self, es=ES, halves=2, calls_per_half=7, blocks_per_call=7,
                 tn=32768, e_total=E, skip_collective=False, skip_gather=False,
                 gq=1, gbufs=2):
        self.skip_collective = skip_collective
        self.skip_gather = skip_gather
        self.gq = gq
        self.gbufs = gbufs
        self.ES = es                        # valid edges per core
        self.H = halves
        self.C = calls_per_half
        self.B = blocks_per_call
        self.BLK = 512
        self.TN = tn                        # local table rows (padded)
        self.E_TOTAL = e_total
        self.HP = self.C * self.B * self.BLK   # padded edges per half
        self.EP = self.H * self.HP             # padded edges per core
        self.HV = es // halves                 # valid edges per half
        assert es % halves == 0
        assert self.HP >= self.HV
        self.NB = self.H * self.C * self.B     # total blocks
        self.NCALL = self.B * self.BLK         # edges per gather call
        self.WCOLS = self.NCALL // 16          # wrapped idx cols per call

    def block_valid(self, b):
        """Valid columns in global block b (rest is padding)."""
        h, lb = divmod(b, self.C * self.B)
        lo = lb * self.BLK
        return int(np.clip(self.HV - lo, 0, self.BLK))


CFG = Cfg()

_PROG_CACHE = {}


def build_program(cfg):
    nc = bacc.Bacc(None, target_bir_lowering=False, num_swdge_queues=4)

    tabs = [nc.declare_dram_parameter(f"tab{h}", [cfg.TN, ND], BF16, isOutput=False)
            for h in range(cfg.H)]
    srcw = nc.declare_dram_parameter("srcw", [128, 2 * cfg.H * cfg.C * cfg.WCOLS], I16, isOutput=False)
    eft = nc.declare_dram_parameter("eft", [ED, cfg.EP], BF16, isOutput=False)
    w1e = nc.declare_dram_parameter("w1e", [ED, OUT], BF16, isOutput=False)
    w1m = nc.declare_dram_parameter("w1m", [ND, OUT], BF16, isOutput=False)
    w2 = nc.declare_dram_parameter("w2", [OUT, OUT], BF16, isOutput=False)
    w3 = nc.declare_dram_parameter("w3", [OUT, OUT], BF16, isOutput=False)
    bias = nc.declare_dram_parameter("bias", [128, 3], F32, isOutput=False)   # b1|b2|b3
    gb = nc.declare_dram_parameter("gb", [128, 2], F32, isOutput=False)       # gamma|beta
    # feature-major output; host transposes back to [EP, OUT]
    out = nc.declare_dram_parameter("out", [OUT, cfg.EP], BF16, isOutput=True)

    LR = mybir.ActivationFunctionType.Lrelu

    with tile.TileContext(nc) as tc:
        with (
            tc.tile_pool(name="singles", bufs=1) as singles,
            tc.tile_pool(name="hres", bufs=1) as hres,
            tc.tile_pool(name="gat", bufs=cfg.gbufs) as gat,
            tc.tile_pool(name="eftp", bufs=2) as eftp,
            tc.tile_pool(name="work", bufs=3) as work,
            tc.tile_pool(name="outp", bufs=3) as outp,
            tc.tile_pool(name="zps", bufs=4, space="PSUM") as zps,
            tc.tile_pool(name="dram", bufs=1, space="DRAM") as dram,
        ):
            # ---- static state ------------------------------------------------
            w1e_t = singles.tile([ED, OUT], BF16)
            nc.sync.dma_start(out=w1e_t[:], in_=w1e[:, :])
            w1m_t = singles.tile([ND, OUT], BF16)
            nc.sync.dma_start(out=w1m_t[:], in_=w1m[:, :])
            w2_t = singles.tile([OUT, OUT], BF16)
            nc.sync.dma_start(out=w2_t[:], in_=w2[:, :])
            w3_t = singles.tile([OUT, OUT], BF16)
            nc.sync.dma_start(out=w3_t[:], in_=w3[:, :])
            bias_t = singles.tile([128, 3], F32)
            nc.sync.dma_start(out=bias_t[:], in_=bias[:, :])
            gb_t = singles.tile([128, 2], F32)
            nc.sync.dma_start(out=gb_t[:], in_=gb[:, :])
            srcw_t = singles.tile([128, 2 * cfg.H * cfg.C * cfg.WCOLS], I16)
            nc.sync.dma_start(out=srcw_t[:], in_=srcw[:, :])

            h3res = hres.tile([128, cfg.EP], BF16)
            stat_stripe = singles.tile([128, cfg.NB, 6], F32)

            # ---- warmup: one tiny gather per queue used (first-use race) -----
            if cfg.gq > 1 and not cfg.skip_gather:
                wsc = singles.tile([128, 1], F32)
                for q in range(cfg.gq):
                    wg = gat.tile([128, 1, 128], BF16, tag="gsd")
                    nc.gpsimd.dma_gather(
                        out_ap=wg[:], in_ap=tabs[0][:, :],
                        idxs_ap=srcw_t[:, 0:8],
                        num_idxs=128, num_idxs_reg=128,
                        elem_size=ND, transpose=True, single_packet=False,
                        queue_num=q,
                        )
                    nc.vector.tensor_reduce(
                        out=wsc[:], in_=wg[:, 0, :],
                        axis=mybir.AxisListType.X, op=mybir.AluOpType.add)

            # ---- phase 1: gather + MLP + stats -------------------------------
            for h in range(cfg.H):
                for c in range(cfg.C):
                    wofs = (h * cfg.C + c) * cfg.WCOLS
                    gsd = gat.tile([128, 1, 2 * cfg.NCALL], BF16, tag="gsd")
                    if cfg.skip_gather:
                        nc.vector.memset(gsd[:], 0.25)
                    else:
                        nc.gpsimd.dma_gather(
                            out_ap=gsd[:], in_ap=tabs[h][:, :],
                            idxs_ap=srcw_t[:, 2 * wofs:2 * (wofs + cfg.WCOLS)],
                            num_idxs=2 * cfg.NCALL, num_idxs_reg=2 * cfg.NCALL,
                            elem_size=ND, transpose=True, single_packet=False,
                            queue_num=(h * cfg.C + c) % cfg.gq,
                        )
                    gs = gsd[:, :, :cfg.NCALL]
                    gd = gsd[:, :, cfg.NCALL:]
                    call_e0 = (h * cfg.C + c) * cfg.NCALL
                    eft_t = eftp.tile([ED, cfg.NCALL], BF16, tag="eft")
                    nc.sync.dma_start(out=eft_t[:], in_=eft[:, call_e0:call_e0 + cfg.NCALL])

                    for b in range(cfg.B):
                        gb_idx = (h * cfg.C + c) * cfg.B + b
                        vb = cfg.block_valid(gb_idx)
                        if vb == 0:
                            continue
                        co = b * cfg.BLK          # col offset within call
                        e0 = call_e0 + co         # global (padded) edge offset
                        zp = zps.tile([128, cfg.BLK], F32, tag="z")
                        nc.tensor.matmul(zp[:, :], lhsT=w1m_t[:], rhs=gs[:, 0, co:co + cfg.BLK],
                                         start=True, stop=False)
                        nc.tensor.matmul(zp[:, :], lhsT=w1m_t[:], rhs=gd[:, 0, co:co + cfg.BLK],
                                         start=False, stop=False)
                        nc.tensor.matmul(zp[:, :], lhsT=w1e_t[:], rhs=eft_t[:, co:co + cfg.BLK],
                                         start=False, stop=True)
                        h1 = work.tile([128, cfg.BLK], BF16, tag="h1")
                        nc.scalar.activation(out=h1[:], in_=zp[:], func=LR,
                                             bias=bias_t[:, 0:1], scale=1.0, alpha=NEG_SLOPE)
                        zp2 = zps.tile([128, cfg.BLK], F32, tag="z")
                        nc.tensor.matmul(zp2[:, :], lhsT=w2_t[:], rhs=h1[:], start=True, stop=True)
                        h2 = work.tile([128, cfg.BLK], BF16, tag="h2")
                        nc.scalar.activation(out=h2[:], in_=zp2[:], func=LR,
                                             bias=bias_t[:, 1:2], scale=1.0, alpha=NEG_SLOPE)
                        zp3 = zps.tile([128, cfg.BLK], F32, tag="z")
                        nc.tensor.matmul(zp3[:, :], lhsT=w3_t[:], rhs=h2[:], start=True, stop=True)
                        h3 = h3res[:, e0:e0 + cfg.BLK]
                        nc.scalar.activation(out=h3[:, :vb], in_=zp3[:, :vb], func=LR,
                                             bias=bias_t[:, 2:3], scale=1.0, alpha=NEG_SLOPE)
                        nc.vector.bn_stats(out=stat_stripe[:, gb_idx, :],
                                           in_=h3[:, :vb])

            # ---- stats + allreduce ------------------------------------------
            # bn_aggr over the valid block stats -> per-core (mean, var)
            nbv = sum(1 for b in range(cfg.NB) if cfg.block_valid(b) > 0)
            # valid blocks are a prefix within each half
            hv_blocks = (cfg.HV + cfg.BLK - 1) // cfg.BLK
            mv = singles.tile([128, 2], F32)
            if cfg.H == 2:
                # stripe slices of the two halves are disjoint; aggregate both
                agg_in = singles.tile([128, 2 * hv_blocks, 6], F32)
                nc.vector.tensor_copy(out=agg_in[:, :hv_blocks, :],
                                      in_=stat_stripe[:, :hv_blocks, :])
                nc.vector.tensor_copy(
                    out=agg_in[:, hv_blocks:, :],
                    in_=stat_stripe[:, cfg.C * cfg.B:cfg.C * cfg.B + hv_blocks, :])
                nc.vector.bn_aggr(out=mv[:], in_=agg_in[:])
            else:
                nc.vector.bn_aggr(out=mv[:], in_=stat_stripe[:, :hv_blocks, :])

            # convert to (sum, sumsq) for the cross-core all-reduce
            st2 = singles.tile([128, 2], F32)
            nc.scalar.mul(out=st2[:, 0:1], in_=mv[:, 0:1], mul=float(cfg.ES))
            msq = singles.tile([128, 1], F32)
            nc.vector.tensor_tensor(out=msq[:], in0=mv[:, 0:1], in1=mv[:, 0:1],
                                    op=mybir.AluOpType.mult)
            nc.vector.tensor_tensor(out=msq[:], in0=mv[:, 1:2], in1=msq[:],
                                    op=mybir.AluOpType.add)
            nc.scalar.mul(out=st2[:, 1:2], in_=msq[:], mul=float(cfg.ES))

            cc_in = dram.tile([128, 2], F32)
            cc_out = dram.tile([128, 2], F32)
            nc.gpsimd.dma_start(out=cc_in[:], in_=st2[:])
            if cfg.skip_collective:
                nc.gpsimd.dma_start(out=cc_out[:], in_=cc_in[:])
            else:
                nc.gpsimd.collective_compute(
                    "AllReduce", mybir.AluOpType.add,
                    replica_groups=[list(range(NCORES))],
                    ins=[cc_in.opt()], outs=[cc_out.opt()],
                )
            gst = singles.tile([128, 2], F32)
            nc.gpsimd.dma_start(out=gst[:], in_=cc_out[:])

            inv_e = 1.0 / cfg.E_TOTAL
            mean_t = singles.tile([128, 1], F32)
            nc.scalar.mul(out=mean_t[:], in_=gst[:, 0:1], mul=inv_e)
            msq_t = singles.tile([128, 1], F32)
            nc.scalar.mul(out=msq_t[:], in_=gst[:, 1:2], mul=inv_e)
            var_t = singles.tile([128, 1], F32)
            nc.vector.tensor_tensor(out=var_t[:], in0=mean_t[:], in1=mean_t[:],
                                    op=mybir.AluOpType.mult)
            nc.vector.tensor_tensor(out=var_t[:], in0=msq_t[:], in1=var_t[:],
                                    op=mybir.AluOpType.subtract)
            eps_t = singles.tile([128, 1], F32)
            nc.vector.memset(eps_t[:], BN_EPS)
            sd_t = singles.tile([128, 1], F32)
            nc.scalar.activation(out=sd_t[:], in_=var_t[:],
                                 func=mybir.ActivationFunctionType.Sqrt,
                                 bias=eps_t[:], scale=1.0)
            rstd_t = singles.tile([128, 1], F32)
            nc.vector.reciprocal(out=rstd_t[:], in_=sd_t[:])
            s_t = singles.tile([128, 1], F32)
            nc.vector.tensor_tensor(out=s_t[:], in0=rstd_t[:], in1=gb_t[:, 0:1],
                                    op=mybir.AluOpType.mult)
            t_t = singles.tile([128, 1], F32)
            nc.vector.tensor_tensor(out=t_t[:], in0=s_t[:], in1=mean_t[:],
                                    op=mybir.AluOpType.mult)
            nc.vector.tensor_tensor(out=t_t[:], in0=gb_t[:, 1:2], in1=t_t[:],
                                    op=mybir.AluOpType.subtract)

            # ---- phase 2: affine + store (feature-major) --------------------
            for gb_idx in range(cfg.NB):
                vb = cfg.block_valid(gb_idx)
                if vb == 0:
                    continue
                e0 = gb_idx * cfg.BLK
                u = outp.tile([128, cfg.BLK], BF16, tag="u")
                nc.vector.tensor_scalar(
                    out=u[:, :vb], in0=h3res[:, e0:e0 + vb],
                    scalar1=s_t[:], scalar2=t_t[:],
                    op0=mybir.AluOpType.mult, op1=mybir.AluOpType.add)
                nc.sync.dma_start(out=out[:, e0:e0 + vb], in_=u[:, :vb])
    nc.compile()
    return nc


def get_program(cfg):
    key = (cfg.ES, cfg.H, cfg.C, cfg.B, cfg.TN, cfg.E_TOTAL,
           cfg.skip_collective, cfg.skip_gather, cfg.gq, cfg.gbufs)
    if key not in _PROG_CACHE:
        _PROG_CACHE[key] = build_program(cfg)
    return _PROG_CACHE[key]


def _wrap_idx(flat, cfg):
    """int16 flat idxs [n] -> wrapped [128, n/16] layout (i at [i%16, i//16], x8)."""
    w = flat.reshape(-1, 16).T.astype(np.int16)      # [16, n/16]
    return np.tile(w, (8, 1))                        # [128, n/16]


def host_prep(node_feats, edge_feats, src, dst, W1, b1, W2, b2, W3, b3, gamma, beta,
              cfg=None):
    cfg = cfg or CFG
    nfb = np.asarray(node_feats, np.float32).astype(ml_dtypes.bfloat16)
    efb = np.asarray(edge_feats, np.float32).astype(ml_dtypes.bfloat16)
    src = np.asarray(src)
    dst = np.asarray(dst)
    W1 = np.asarray(W1, np.float32)

    w1e = W1[:ED].astype(ml_dtypes.bfloat16)
    w1m = W1[ED:].astype(ml_dtypes.bfloat16)
    w2b = np.asarray(W2, np.float32).astype(ml_dtypes.bfloat16)
    w3b = np.asarray(W3, np.float32).astype(ml_dtypes.bfloat16)
    bias = np.stack([np.asarray(b1, np.float32),
                     np.asarray(b2, np.float32),
                     np.asarray(b3, np.float32)], axis=1)          # [128, 3]
    gb = np.stack([np.asarray(gamma, np.float32),
                   np.asarray(beta, np.float32)], axis=1)          # [128, 2]

    in_maps = []
    for c in range(NCORES):
        base = c * cfg.ES
        tabs, sws = [], []
        for h in range(cfg.H):
            lo = base + h * cfg.HV
            s_h = src[lo:lo + cfg.HV]
            d_h = dst[lo:lo + cfg.HV]
            u = np.unique(np.concatenate([s_h, d_h]))
            assert len(u) <= cfg.TN, f"local table overflow: {len(u)} > {cfg.TN}"
            assert len(u) <= 32768, "int16 index overflow"
            tab = np.zeros((cfg.TN, ND), ml_dtypes.bfloat16)
            tab[:len(u)] = nfb[u]
            tabs.append(tab)
            s16 = np.searchsorted(u, s_h).astype(np.int16)
            d16 = np.searchsorted(u, d_h).astype(np.int16)
            pad = cfg.HP - cfg.HV
            if pad:
                s16 = np.concatenate([s16, np.zeros(pad, np.int16)])
                d16 = np.concatenate([d16, np.zeros(pad, np.int16)])
            # one wrapped array per gather call: src block then dst block merged
            for cl in range(cfg.C):
                sws.append(_wrap_idx(s16[cl * cfg.NCALL:(cl + 1) * cfg.NCALL], cfg))
                sws.append(_wrap_idx(d16[cl * cfg.NCALL:(cl + 1) * cfg.NCALL], cfg))
        srcw = np.concatenate(sws, axis=1)
        # edge feats, transposed + per-half padding
        eftc = np.zeros((ED, cfg.EP), ml_dtypes.bfloat16)
        for h in range(cfg.H):
            lo = base + h * cfg.HV
            eftc[:, h * cfg.HP:h * cfg.HP + cfg.HV] = efb[lo:lo + cfg.HV].T
        im = {"srcw": srcw, "eft": eftc,
              "w1e": w1e, "w1m": w1m, "w2": w2b, "w3": w3b,
              "bias": bias, "gb": gb}
        for h in range(cfg.H):
            im[f"tab{h}"] = tabs[h]
        in_maps.append(im)
    return in_maps


def assemble_output(results, cfg=None):
    cfg = cfg or CFG
    out = np.empty((NCORES * cfg.ES, OUT), np.float32)
    for c in range(NCORES):
        oc = np.asarray(results[c]["out"]).astype(np.float32)   # [OUT, EP]
        for h in range(cfg.H):
            lo = c * cfg.ES + h * cfg.HV
            out[lo:lo + cfg.HV] = oc[:, h * cfg.HP:h * cfg.HP + cfg.HV].T
    return out


def kernel(**inputs):
    cfg = CFG
    nc = get_program(cfg)
    in_maps = host_prep(**inputs, cfg=cfg)
    res = run_bass_kernel_spmd(nc, in_maps, list(range(NCORES)))
    return assemble_output(res.results, cfg)


# revision 14
# speedup vs baseline: 1.0913x; 1.0419x over previous
"""Trainium2 Bass kernel for nn_CysInteractLayer (GNN message-passing layer).

out = BN(lrelu(lrelu(lrelu([ef | nf[src]+nf[dst]] @ W1 + b1) @ W2 + b2) @ W3 + b3))

Sharding: edges across 8 NeuronCores; node_feats/weights replicated
(as per-half local node tables so dma_gather's int16 indices suffice);
BN batch stats all-reduced across cores on-device.

v2: feature-major bf16 output (host un-transposes; removes on-chip PE
transposes), bn_stats/bn_aggr for BN statistics, DVE affine in phase 2,
optional multi-queue gathers (gq>1) with per-queue warmup.
"""
import numpy as np
import ml_dtypes

import concourse.bass as bass
import concourse.bacc as bacc
import concourse.tile as tile
from concourse import mybir
from concourse.bass_utils import run_bass_kernel_spmd

F32 = mybir.dt.float32
BF16 = mybir.dt.bfloat16
I16 = mybir.dt.int16

# problem constants (hardcoded per harness contract)
V, E = 50000, 400000
ND, ED, OUT = 128, 64, 128
IN = ND + ED
NEG_SLOPE = 0.01
BN_EPS = 1e-5

NCORES = 8
ES = E // NCORES            # 50000 edges per core


class Cfg:
    """Geometry of the per-core kernel; small instances used for sim tests."""

    def __init__(self, es=ES, halves=2, calls_per_half=7, blocks_per_call=7,
                 tn=32768, e_total=E, skip_collective=False, skip_gather=False,
                 gq=1, gbufs=2):
        self.skip_collective = skip_collective
        self.skip_gather = skip_gather
        self.gq = gq
        self.gbufs = gbufs
        self.ES = es                        # valid edges per core
        self.H = halves
        self.C = calls_per_half
        self.B = blocks_per_call
        self.BLK = 512
        self.TN = tn                        # local table rows (padded)
        self.E_TOTAL = e_total
        self.HP = self.C * self.B * self.BLK   # padded edges per half
        self.EP = self.H * self.HP             # padded edges per core
        self.HV = es // halves                 # valid edges per half
        assert es % halves == 0
        assert self.HP >= self.HV
        self.NB = self.H * self.C * self.B     # total blocks
        self.NCALL = self.B * self.BLK         # edges per gather call
        self.WCOLS = self.NCALL // 16          # wrapped idx cols per call

    def block_valid(self, b):
        """Valid columns in global block b (rest is padding)."""
        h, lb = divmod(b, self.C * self.B)
        lo = lb * self.BLK
        return int(np.clip(self.HV - lo, 0, self.BLK))


CFG = Cfg()

_PROG_CACHE = {}


def build_program(cfg):
    nc = bacc.Bacc(None, target_bir_lowering=False, num_swdge_queues=4)

    tabs = [nc.declare_dram_parameter(f"tab{h}", [cfg.TN, ND], BF16, isOutput=False)
            for h in range(cfg.H)]
    srcw = nc.declare_dram_parameter("srcw", [128, 2 * cfg.H * cfg.C * cfg.WCOLS], I16, isOutput=False)
    eft = nc.declare_dram_parameter("eft", [ED, cfg.EP], BF16, isOutput=False)
    w1e = nc.declare_dram_parameter("w1e", [ED, OUT], BF16, isOutput=False)
    w1m = nc.declare_dram_parameter("w1m", [ND, OUT], BF16, isOutput=False)
    w2 = nc.declare_dram_parameter("w2", [OUT, OUT], BF16, isOutput=False)
    w3 = nc.declare_dram_parameter("w3", [OUT, OUT], BF16, isOutput=False)
    bias = nc.declare_dram_parameter("bias", [128, 3], F32, isOutput=False)   # b1|b2|b3
    gb = nc.declare_dram_parameter("gb", [128, 2], F32, isOutput=False)       # gamma|beta
    # feature-major output; host transposes back to [EP, OUT]
    out = nc.declare_dram_parameter("out", [OUT, cfg.EP], BF16, isOutput=True)

    LR = mybir.ActivationFunctionType.Lrelu

    with tile.TileContext(nc) as tc:
        with (
            tc.tile_pool(name="singles", bufs=1) as singles,
            tc.tile_pool(name="hres", bufs=1) as hres,
            tc.tile_pool(name="gat", bufs=cfg.gbufs) as gat,
            tc.tile_pool(name="eftp", bufs=2) as eftp,
            tc.tile_pool(name="work", bufs=3) as work,
            tc.tile_pool(name="outp", bufs=2) as outp,
            tc.tile_pool(name="zps", bufs=4, space="PSUM") as zps,
            tc.tile_pool(name="dram", bufs=1, space="DRAM") as dram,
        ):
            # ---- static state (srcw first: gathers depend on it) -------------
            srcw_t = singles.tile([128, 2 * cfg.H * cfg.C * cfg.WCOLS], I16)
            nc.sync.dma_start(out=srcw_t[:], in_=srcw[:, :])
            w1e_t = singles.tile([ED, OUT], BF16)
            nc.sync.dma_start(out=w1e_t[:], in_=w1e[:, :])
            w1m_t = singles.tile([ND, OUT], BF16)
            nc.sync.dma_start(out=w1m_t[:], in_=w1m[:, :])
            w2_t = singles.tile([OUT, OUT], BF16)
            nc.sync.dma_start(out=w2_t[:], in_=w2[:, :])
            w3_t = singles.tile([OUT, OUT], BF16)
            nc.sync.dma_start(out=w3_t[:], in_=w3[:, :])
            bias_t = singles.tile([128, 3], F32)
            nc.sync.dma_start(out=bias_t[:], in_=bias[:, :])
            gb_t = singles.tile([128, 2], F32)
            nc.sync.dma_start(out=gb_t[:], in_=gb[:, :])

            h3res = hres.tile([128, cfg.EP], BF16)
            stat_stripe = singles.tile([128, cfg.NB, 6], F32)

            # ---- warmup: one tiny gather per queue used (first-use race) -----
            if cfg.gq > 1 and not cfg.skip_gather:
                wsc = singles.tile([128, 1], F32)
                for q in range(cfg.gq):
                    wg = gat.tile([128, 1, 128], BF16, tag="gsd")
                    nc.gpsimd.dma_gather(
                        out_ap=wg[:], in_ap=tabs[0][:, :],
                        idxs_ap=srcw_t[:, 0:8],
                        num_idxs=128, num_idxs_reg=128,
                        elem_size=ND, transpose=True, single_packet=False,
                        queue_num=q,
                        )
                    nc.vector.tensor_reduce(
                        out=wsc[:], in_=wg[:, 0, :],
                        axis=mybir.AxisListType.X, op=mybir.AluOpType.add)

            # ---- phase 1: gather + MLP + stats -------------------------------
            for h in range(cfg.H):
                for c in range(cfg.C):
                    wofs = (h * cfg.C + c) * cfg.WCOLS
                    gsd = gat.tile([128, 1, 2 * cfg.NCALL], BF16, tag="gsd")
                    if cfg.skip_gather:
                        nc.vector.memset(gsd[:], 0.25)
                    else:
                        nc.gpsimd.dma_gather(
                            out_ap=gsd[:], in_ap=tabs[h][:, :],
                            idxs_ap=srcw_t[:, 2 * wofs:2 * (wofs + cfg.WCOLS)],
                            num_idxs=2 * cfg.NCALL, num_idxs_reg=2 * cfg.NCALL,
                            elem_size=ND, transpose=True, single_packet=False,
                            queue_num=(h * cfg.C + c) % cfg.gq,
                        )
                    gs = gsd[:, :, :cfg.NCALL]
                    gd = gsd[:, :, cfg.NCALL:]
                    call_e0 = (h * cfg.C + c) * cfg.NCALL
                    eft_t = eftp.tile([ED, cfg.NCALL], BF16, tag="eft")
                    nc.sync.dma_start(out=eft_t[:], in_=eft[:, call_e0:call_e0 + cfg.NCALL])

                    for b in range(cfg.B):
                        gb_idx = (h * cfg.C + c) * cfg.B + b
                        vb = cfg.block_valid(gb_idx)
                        if vb == 0:
                            continue
                        co = b * cfg.BLK          # col offset within call
                        e0 = call_e0 + co         # global (padded) edge offset
                        zp = zps.tile([128, cfg.BLK], F32, tag="z")
                        nc.tensor.matmul(zp[:, :], lhsT=w1m_t[:], rhs=gs[:, 0, co:co + cfg.BLK],
                                         start=True, stop=False)
                        nc.tensor.matmul(zp[:, :], lhsT=w1m_t[:], rhs=gd[:, 0, co:co + cfg.BLK],
                                         start=False, stop=False)
                        nc.tensor.matmul(zp[:, :], lhsT=w1e_t[:], rhs=eft_t[:, co:co + cfg.BLK],
                                         start=False, stop=True)
                        h1 = work.tile([128, cfg.BLK], BF16, tag="h1")
                        nc.scalar.activation(out=h1[:], in_=zp[:], func=LR,
                                             bias=bias_t[:, 0:1], scale=1.0, alpha=NEG_SLOPE)
                        zp2 = zps.tile([128, cfg.BLK], F32, tag="z")
                        nc.tensor.matmul(zp2[:, :], lhsT=w2_t[:], rhs=h1[:], start=True, stop=True)
                        h2 = work.tile([128, cfg.BLK], BF16, tag="h2")
                        nc.scalar.activation(out=h2[:], in_=zp2[:], func=LR,
                                             bias=bias_t[:, 1:2], scale=1.0, alpha=NEG_SLOPE)
                        zp3 = zps.tile([128, cfg.BLK], F32, tag="z")
                        nc.tensor.matmul(zp3[:, :], lhsT=w3_t[:], rhs=h2[:], start=True, stop=True)
                        h3 = h3res[:, e0:e0 + cfg.BLK]
                        nc.scalar.activation(out=h3[:, :vb], in_=zp3[:, :vb], func=LR,
                                             bias=bias_t[:, 2:3], scale=1.0, alpha=NEG_SLOPE)
                        nc.vector.bn_stats(out=stat_stripe[:, gb_idx, :],
                                           in_=h3[:, :vb])

            # ---- stats + allreduce ------------------------------------------
            hv_blocks = (cfg.HV + cfg.BLK - 1) // cfg.BLK
            mv = singles.tile([128, 2], F32)
            if cfg.H == 2:
                agg_in = singles.tile([128, 2 * hv_blocks, 6], F32)
                nc.vector.tensor_copy(out=agg_in[:, :hv_blocks, :],
                                      in_=stat_stripe[:, :hv_blocks, :])
                nc.vector.tensor_copy(
                    out=agg_in[:, hv_blocks:, :],
                    in_=stat_stripe[:, cfg.C * cfg.B:cfg.C * cfg.B + hv_blocks, :])
                nc.vector.bn_aggr(out=mv[:], in_=agg_in[:])
            else:
                nc.vector.bn_aggr(out=mv[:], in_=stat_stripe[:, :hv_blocks, :])

            st2 = singles.tile([128, 2], F32)
            nc.scalar.mul(out=st2[:, 0:1], in_=mv[:, 0:1], mul=float(cfg.ES))
            msq = singles.tile([128, 1], F32)
            nc.vector.tensor_tensor(out=msq[:], in0=mv[:, 0:1], in1=mv[:, 0:1],
                                    op=mybir.AluOpType.mult)
            nc.vector.tensor_tensor(out=msq[:], in0=mv[:, 1:2], in1=msq[:],
                                    op=mybir.AluOpType.add)
            nc.scalar.mul(out=st2[:, 1:2], in_=msq[:], mul=float(cfg.ES))

            cc_in = dram.tile([128, 2], F32)
            cc_out = dram.tile([128, 2], F32)
            nc.gpsimd.dma_start(out=cc_in[:], in_=st2[:])
            if cfg.skip_collective:
                nc.gpsimd.dma_start(out=cc_out[:], in_=cc_in[:])
            else:
                nc.gpsimd.collective_compute(
                    "AllReduce", mybir.AluOpType.add,
                    replica_groups=[list(range(NCORES))],
                    ins=[cc_in.opt()], outs=[cc_out.opt()],
                )
            gst = singles.tile([128, 2], F32)
            nc.gpsimd.dma_start(out=gst[:], in_=cc_out[:])

            inv_e = 1.0 / cfg.E_TOTAL
            mean_t = singles.tile([128, 1], F32)
            nc.scalar.mul(out=mean_t[:], in_=gst[:, 0:1], mul=inv_e)
            msq_t = singles.tile([128, 1], F32)
            nc.scalar.mul(out=msq_t[:], in_=gst[:, 1:2], mul=inv_e)
            var_t = singles.tile([128, 1], F32)
            nc.vector.tensor_tensor(out=var_t[:], in0=mean_t[:], in1=mean_t[:],
                                    op=mybir.AluOpType.mult)
            nc.vector.tensor_tensor(out=var_t[:], in0=msq_t[:], in1=var_t[:],
                                    op=mybir.AluOpType.subtract)
            eps_t = singles.tile([128, 1], F32)
            nc.vector.memset(eps_t[:], BN_EPS)
            sd_t = singles.tile([128, 1], F32)
            nc.scalar.activation(out=sd_t[:], in_=var_t[:],
                                 func=mybir.ActivationFunctionType.Sqrt,
                                 bias=eps_t[:], scale=1.0)
            rstd_t = singles.tile([128, 1], F32)
            nc.vector.reciprocal(out=rstd_t[:], in_=sd_t[:])
            s_t = singles.tile([128, 1], F32)
            nc.vector.tensor_tensor(out=s_t[:], in0=rstd_t[:], in1=gb_t[:, 0:1],
                                    op=mybir.AluOpType.mult)
            t_t = singles.tile([128, 1], F32)
            nc.vector.tensor_tensor(out=t_t[:], in0=s_t[:], in1=mean_t[:],
                                    op=mybir.AluOpType.mult)
            nc.vector.tensor_tensor(out=t_t[:], in0=gb_t[:, 1:2], in1=t_t[:],
                                    op=mybir.AluOpType.subtract)

            # ---- phase 2: affine + store (feature-major, call-sized tiles) --
            for cg in range(cfg.H * cfg.C):
                e0 = cg * cfg.NCALL
                vb = min(max(cfg.HV - (cg % cfg.C) * cfg.NCALL, 0), cfg.NCALL)
                if vb == 0:
                    continue
                u = outp.tile([128, cfg.NCALL], BF16, tag="u")
                nc.vector.tensor_scalar(
                    out=u[:, :vb], in0=h3res[:, e0:e0 + vb],
                    scalar1=s_t[:], scalar2=t_t[:],
                    op0=mybir.AluOpType.mult, op1=mybir.AluOpType.add)
                nc.sync.dma_start(out=out[:, e0:e0 + vb], in_=u[:, :vb])
    nc.compile()
    return nc


def get_program(cfg):
    key = (cfg.ES, cfg.H, cfg.C, cfg.B, cfg.TN, cfg.E_TOTAL,
           cfg.skip_collective, cfg.skip_gather, cfg.gq, cfg.gbufs)
    if key not in _PROG_CACHE:
        _PROG_CACHE[key] = build_program(cfg)
    return _PROG_CACHE[key]


def _wrap_idx(flat, cfg):
    """int16 flat idxs [n] -> wrapped [128, n/16] layout (i at [i%16, i//16], x8)."""
    w = flat.reshape(-1, 16).T.astype(np.int16)      # [16, n/16]
    return np.tile(w, (8, 1))                        # [128, n/16]


def host_prep(node_feats, edge_feats, src, dst, W1, b1, W2, b2, W3, b3, gamma, beta,
              cfg=None):
    cfg = cfg or CFG
    nfb = np.asarray(node_feats, np.float32).astype(ml_dtypes.bfloat16)
    efb = np.asarray(edge_feats, np.float32).astype(ml_dtypes.bfloat16)
    src = np.asarray(src)
    dst = np.asarray(dst)
    W1 = np.asarray(W1, np.float32)

    w1e = W1[:ED].astype(ml_dtypes.bfloat16)
    w1m = W1[ED:].astype(ml_dtypes.bfloat16)
    w2b = np.asarray(W2, np.float32).astype(ml_dtypes.bfloat16)
    w3b = np.asarray(W3, np.float32).astype(ml_dtypes.bfloat16)
    bias = np.stack([np.asarray(b1, np.float32),
                     np.asarray(b2, np.float32),
                     np.asarray(b3, np.float32)], axis=1)          # [128, 3]
    gb = np.stack([np.asarray(gamma, np.float32),
                   np.asarray(beta, np.float32)], axis=1)          # [128, 2]

    in_maps = []
    for c in range(NCORES):
        base = c * cfg.ES
        tabs, sws = [], []
        for h in range(cfg.H):
            lo = base + h * cfg.HV
            s_h = src[lo:lo + cfg.HV]
            d_h = dst[lo:lo + cfg.HV]
            u = np.unique(np.concatenate([s_h, d_h]))
            assert len(u) <= cfg.TN, f"local table overflow: {len(u)} > {cfg.TN}"
            assert len(u) <= 32768, "int16 index overflow"
            tab = np.zeros((cfg.TN, ND), ml_dtypes.bfloat16)
            tab[:len(u)] = nfb[u]
            tabs.append(tab)
            s16 = np.searchsorted(u, s_h).astype(np.int16)
            d16 = np.searchsorted(u, d_h).astype(np.int16)
            pad = cfg.HP - cfg.HV
            if pad:
                s16 = np.concatenate([s16, np.zeros(pad, np.int16)])
                d16 = np.concatenate([d16, np.zeros(pad, np.int16)])
            # one wrapped array per gather call: src block then dst block merged
            for cl in range(cfg.C):
                sws.append(_wrap_idx(s16[cl * cfg.NCALL:(cl + 1) * cfg.NCALL], cfg))
                sws.append(_wrap_idx(d16[cl * cfg.NCALL:(cl + 1) * cfg.NCALL], cfg))
        srcw = np.concatenate(sws, axis=1)
        # edge feats, transposed + per-half padding
        eftc = np.zeros((ED, cfg.EP), ml_dtypes.bfloat16)
        for h in range(cfg.H):
            lo = base + h * cfg.HV
            eftc[:, h * cfg.HP:h * cfg.HP + cfg.HV] = efb[lo:lo + cfg.HV].T
        im = {"srcw": srcw, "eft": eftc,
              "w1e": w1e, "w1m": w1m, "w2": w2b, "w3": w3b,
              "bias": bias, "gb": gb}
        for h in range(cfg.H):
            im[f"tab{h}"] = tabs[h]
        in_maps.append(im)
    return in_maps


def assemble_output(results, cfg=None):
    cfg = cfg or CFG
    out = np.empty((NCORES * cfg.ES, OUT), np.float32)
    for c in range(NCORES):
        oc = np.asarray(results[c]["out"]).astype(np.float32)   # [OUT, EP]
        for h in range(cfg.H):
            lo = c * cfg.ES + h * cfg.HV
            out[lo:lo + cfg.HV] = oc[:, h * cfg.HP:h * cfg.HP + cfg.HV].T
    return out


def kernel(**inputs):
    cfg = CFG
    nc = get_program(cfg)
    in_maps = host_prep(**inputs, cfg=cfg)
    res = run_bass_kernel_spmd(nc, in_maps, list(range(NCORES)))
    return assemble_output(res.results, cfg)


# revision 17
# speedup vs baseline: 1.6778x; 1.5374x over previous
"""Trainium2 Bass kernel for nn_CysInteractLayer (GNN message-passing layer).

out = BN(lrelu(lrelu(lrelu([ef | nf[src]+nf[dst]] @ W1 + b1) @ W2 + b2) @ W3 + b3))

Sharding: edges across 8 NeuronCores; node_feats/weights replicated
(as per-half local node tables so dma_gather's int16 indices suffice);
BN batch stats all-reduced across cores on-device.

v2: feature-major bf16 output (host un-transposes; removes on-chip PE
transposes), bn_stats/bn_aggr for BN statistics, DVE affine in phase 2,
optional multi-queue gathers (gq>1) with per-queue warmup.
"""
import numpy as np
import ml_dtypes

import concourse.bass as bass
import concourse.bacc as bacc
import concourse.tile as tile
from concourse import mybir
from concourse.bass_utils import run_bass_kernel_spmd

F32 = mybir.dt.float32
BF16 = mybir.dt.bfloat16
I16 = mybir.dt.int16

# problem constants (hardcoded per harness contract)
V, E = 50000, 400000
ND, ED, OUT = 128, 64, 128
IN = ND + ED
NEG_SLOPE = 0.01
BN_EPS = 1e-5

NCORES = 8
ES = E // NCORES            # 50000 edges per core


class Cfg:
    """Geometry of the per-core kernel; small instances used for sim tests."""

    def __init__(self, es=ES, halves=2, calls_per_half=7, blocks_per_call=7,
                 tn=32768, e_total=E, skip_collective=False, skip_gather=False,
                 gq=1, gbufs=3, banded=True):
        self.banded = banded
        self.skip_collective = skip_collective
        self.skip_gather = skip_gather
        self.gq = gq
        self.gbufs = gbufs
        self.ES = es                        # valid edges per core
        self.H = halves
        self.C = calls_per_half
        self.B = blocks_per_call
        self.BLK = 512
        self.TN = tn                        # local table rows (padded)
        self.E_TOTAL = e_total
        self.HP = self.C * self.B * self.BLK   # padded edges per half
        self.EP = self.H * self.HP             # padded edges per core
        self.HV = es // halves                 # valid edges per half
        assert es % halves == 0
        assert self.HP >= self.HV
        self.NB = self.H * self.C * self.B     # total blocks
        self.NCALL = self.B * self.BLK         # edges per gather call
        self.WCOLS = self.NCALL // 16          # wrapped idx cols per call

    def block_valid(self, b):
        """Valid columns in global block b (rest is padding)."""
        h, lb = divmod(b, self.C * self.B)
        lo = lb * self.BLK
        return int(np.clip(self.HV - lo, 0, self.BLK))


CFG = Cfg()

_PROG_CACHE = {}


def build_program(cfg):
    nc = bacc.Bacc(None, target_bir_lowering=False, num_swdge_queues=4)

    tabs = [nc.declare_dram_parameter(f"tab{h}", [cfg.TN, ND], BF16, isOutput=False)
            for h in range(cfg.H)]
    nidx = (1 if cfg.banded else 2)
    srcw = nc.declare_dram_parameter("srcw", [128, nidx * cfg.H * cfg.C * cfg.WCOLS], I16, isOutput=False)
    if cfg.banded:
        btab = nc.declare_dram_parameter("btab", [cfg.NB * 128, ND], BF16, isOutput=False)
        offr = nc.declare_dram_parameter("offr", [1, cfg.EP], BF16, isOutput=False)
        iot = nc.declare_dram_parameter("iot", [128, 1], F32, isOutput=False)
        ones1 = nc.declare_dram_parameter("ones1", [1, 128], BF16, isOutput=False)
    eft = nc.declare_dram_parameter("eft", [ED, cfg.EP], BF16, isOutput=False)
    w1e = nc.declare_dram_parameter("w1e", [ED, OUT], BF16, isOutput=False)
    w1m = nc.declare_dram_parameter("w1m", [ND, OUT], BF16, isOutput=False)
    w2 = nc.declare_dram_parameter("w2", [OUT, OUT], BF16, isOutput=False)
    w3 = nc.declare_dram_parameter("w3", [OUT, OUT], BF16, isOutput=False)
    bias = nc.declare_dram_parameter("bias", [128, 3], F32, isOutput=False)   # b1|b2|b3
    gb = nc.declare_dram_parameter("gb", [128, 2], F32, isOutput=False)       # gamma|beta
    # feature-major output; host transposes back to [EP, OUT]
    out = nc.declare_dram_parameter("out", [OUT, cfg.EP], BF16, isOutput=True)

    LR = mybir.ActivationFunctionType.Lrelu

    with tile.TileContext(nc) as tc:
        with (
            tc.tile_pool(name="singles", bufs=1) as singles,
            tc.tile_pool(name="hres", bufs=1) as hres,
            tc.tile_pool(name="gat", bufs=cfg.gbufs) as gat,
            tc.tile_pool(name="eftp", bufs=2) as eftp,
            tc.tile_pool(name="work", bufs=3) as work,
            tc.tile_pool(name="outp", bufs=2) as outp,
            tc.tile_pool(name="zps", bufs=4, space="PSUM") as zps,
            tc.tile_pool(name="dram", bufs=1, space="DRAM") as dram,
        ):
            # ---- static state (srcw first: gathers depend on it) -------------
            srcw_t = singles.tile([128, nidx * cfg.H * cfg.C * cfg.WCOLS], I16)
            nc.sync.dma_start(out=srcw_t[:], in_=srcw[:, :])
            if cfg.banded:
                iot_t = singles.tile([128, 1], F32)
                nc.sync.dma_start(out=iot_t[:], in_=iot[:, :])
                ones1_t = singles.tile([1, 128], BF16)
                nc.sync.dma_start(out=ones1_t[:], in_=ones1[:, :])
            w1e_t = singles.tile([ED, OUT], BF16)
            nc.sync.dma_start(out=w1e_t[:], in_=w1e[:, :])
            w1m_t = singles.tile([ND, OUT], BF16)
            nc.sync.dma_start(out=w1m_t[:], in_=w1m[:, :])
            w2_t = singles.tile([OUT, OUT], BF16)
            nc.sync.dma_start(out=w2_t[:], in_=w2[:, :])
            w3_t = singles.tile([OUT, OUT], BF16)
            nc.sync.dma_start(out=w3_t[:], in_=w3[:, :])
            bias_t = singles.tile([128, 3], F32)
            nc.sync.dma_start(out=bias_t[:], in_=bias[:, :])
            gb_t = singles.tile([128, 2], F32)
            nc.sync.dma_start(out=gb_t[:], in_=gb[:, :])

            h3res = hres.tile([128, cfg.EP], BF16)
            stat_stripe = singles.tile([128, cfg.NB, 6], F32)

            # ---- warmup: one tiny gather per queue used (first-use race) -----
            if cfg.gq > 1 and not cfg.skip_gather:
                wsc = singles.tile([128, 1], F32)
                for q in range(cfg.gq):
                    wg = gat.tile([128, 1, 128], BF16, tag="gsd")
                    nc.gpsimd.dma_gather(
                        out_ap=wg[:], in_ap=tabs[0][:, :],
                        idxs_ap=srcw_t[:, 0:8],
                        num_idxs=128, num_idxs_reg=128,
                        elem_size=ND, transpose=True, single_packet=False,
                        queue_num=q,
                        )
                    nc.vector.tensor_reduce(
                        out=wsc[:], in_=wg[:, 0, :],
                        axis=mybir.AxisListType.X, op=mybir.AluOpType.add)

            # ---- phase 1: gather + MLP + stats -------------------------------
            for h in range(cfg.H):
                for c in range(cfg.C):
                    wofs = (h * cfg.C + c) * cfg.WCOLS
                    gsd = gat.tile([128, 1, nidx * cfg.NCALL], BF16, tag="gsd")
                    if cfg.skip_gather:
                        nc.vector.memset(gsd[:], 0.25)
                    else:
                        nc.gpsimd.dma_gather(
                            out_ap=gsd[:], in_ap=tabs[h][:, :],
                            idxs_ap=srcw_t[:, nidx * wofs:nidx * (wofs + cfg.WCOLS)],
                            num_idxs=nidx * cfg.NCALL, num_idxs_reg=nidx * cfg.NCALL,
                            elem_size=ND, transpose=True, single_packet=False,
                            queue_num=(h * cfg.C + c) % cfg.gq,
                        )
                    if cfg.banded:
                        gd = gsd[:, :, :cfg.NCALL]
                    else:
                        gs = gsd[:, :, :cfg.NCALL]
                        gd = gsd[:, :, cfg.NCALL:]
                    call_e0 = (h * cfg.C + c) * cfg.NCALL
                    eft_t = eftp.tile([ED, cfg.NCALL], BF16, tag="eft")
                    nc.sync.dma_start(out=eft_t[:], in_=eft[:, call_e0:call_e0 + cfg.NCALL])
                    if cfg.banded:
                        offc_t = eftp.tile([1, cfg.NCALL], BF16, tag="offc")
                        nc.sync.dma_start(out=offc_t[:], in_=offr[0:1, call_e0:call_e0 + cfg.NCALL])

                    for b in range(cfg.B):
                        gb_idx = (h * cfg.C + c) * cfg.B + b
                        vb = cfg.block_valid(gb_idx)
                        if vb == 0:
                            continue
                        co = b * cfg.BLK          # col offset within call
                        e0 = call_e0 + co         # global (padded) edge offset
                        zp = zps.tile([128, cfg.BLK], F32, tag="z")
                        if cfg.banded:
                            zoff = zps.tile([128, cfg.BLK], F32, tag="zoff")
                            nc.tensor.matmul(zoff[:, :], lhsT=ones1_t[:],
                                             rhs=offc_t[0:1, co:co + cfg.BLK],
                                             start=True, stop=True)
                            oh = work.tile([128, cfg.BLK], BF16, tag="oh")
                            nc.vector.tensor_scalar(
                                out=oh[:], in0=zoff[:], scalar1=iot_t[:], scalar2=None,
                                op0=mybir.AluOpType.is_equal)
                            bt = eftp.tile([128, ND], BF16, tag="bt")
                            nc.sync.dma_start(
                                out=bt[:], in_=btab[gb_idx * 128:(gb_idx + 1) * 128, :])
                            nc.tensor.matmul(zp[:, :], lhsT=bt[:], rhs=oh[:],
                                             start=True, stop=False)
                        else:
                            nc.tensor.matmul(zp[:, :], lhsT=w1m_t[:], rhs=gs[:, 0, co:co + cfg.BLK],
                                             start=True, stop=False)
                        nc.tensor.matmul(zp[:, :], lhsT=w1m_t[:], rhs=gd[:, 0, co:co + cfg.BLK],
                                         start=False, stop=False)
                        nc.tensor.matmul(zp[:, :], lhsT=w1e_t[:], rhs=eft_t[:, co:co + cfg.BLK],
                                         start=False, stop=True)
                        h1 = work.tile([128, cfg.BLK], BF16, tag="h1")
                        nc.scalar.activation(out=h1[:], in_=zp[:], func=LR,
                                             bias=bias_t[:, 0:1], scale=1.0, alpha=NEG_SLOPE)
                        zp2 = zps.tile([128, cfg.BLK], F32, tag="z")
                        nc.tensor.matmul(zp2[:, :], lhsT=w2_t[:], rhs=h1[:], start=True, stop=True)
                        h2 = work.tile([128, cfg.BLK], BF16, tag="h2")
                        nc.scalar.activation(out=h2[:], in_=zp2[:], func=LR,
                                             bias=bias_t[:, 1:2], scale=1.0, alpha=NEG_SLOPE)
                        zp3 = zps.tile([128, cfg.BLK], F32, tag="z")
                        nc.tensor.matmul(zp3[:, :], lhsT=w3_t[:], rhs=h2[:], start=True, stop=True)
                        h3 = h3res[:, e0:e0 + cfg.BLK]
                        nc.scalar.activation(out=h3[:, :vb], in_=zp3[:, :vb], func=LR,
                                             bias=bias_t[:, 2:3], scale=1.0, alpha=NEG_SLOPE)
                        nc.vector.bn_stats(out=stat_stripe[:, gb_idx, :],
                                           in_=h3[:, :vb])

            # ---- stats + allreduce ------------------------------------------
            hv_blocks = (cfg.HV + cfg.BLK - 1) // cfg.BLK
            mv = singles.tile([128, 2], F32)
            if cfg.H == 2:
                agg_in = singles.tile([128, 2 * hv_blocks, 6], F32)
                nc.vector.tensor_copy(out=agg_in[:, :hv_blocks, :],
                                      in_=stat_stripe[:, :hv_blocks, :])
                nc.vector.tensor_copy(
                    out=agg_in[:, hv_blocks:, :],
                    in_=stat_stripe[:, cfg.C * cfg.B:cfg.C * cfg.B + hv_blocks, :])
                nc.vector.bn_aggr(out=mv[:], in_=agg_in[:])
            else:
                nc.vector.bn_aggr(out=mv[:], in_=stat_stripe[:, :hv_blocks, :])

            st2 = singles.tile([128, 2], F32)
            nc.scalar.mul(out=st2[:, 0:1], in_=mv[:, 0:1], mul=float(cfg.ES))
            msq = singles.tile([128, 1], F32)
            nc.vector.tensor_tensor(out=msq[:], in0=mv[:, 0:1], in1=mv[:, 0:1],
                                    op=mybir.AluOpType.mult)
            nc.vector.tensor_tensor(out=msq[:], in0=mv[:, 1:2], in1=msq[:],
                                    op=mybir.AluOpType.add)
            nc.scalar.mul(out=st2[:, 1:2], in_=msq[:], mul=float(cfg.ES))

            cc_in = dram.tile([128, 2], F32)
            cc_out = dram.tile([128, 2], F32)
            nc.gpsimd.dma_start(out=cc_in[:], in_=st2[:])
            if cfg.skip_collective:
                nc.gpsimd.dma_start(out=cc_out[:], in_=cc_in[:])
            else:
                nc.gpsimd.collective_compute(
                    "AllReduce", mybir.AluOpType.add,
                    replica_groups=[list(range(NCORES))],
                    ins=[cc_in.opt()], outs=[cc_out.opt()],
                )
            gst = singles.tile([128, 2], F32)
            nc.gpsimd.dma_start(out=gst[:], in_=cc_out[:])

            inv_e = 1.0 / cfg.E_TOTAL
            mean_t = singles.tile([128, 1], F32)
            nc.scalar.mul(out=mean_t[:], in_=gst[:, 0:1], mul=inv_e)
            msq_t = singles.tile([128, 1], F32)
            nc.scalar.mul(out=msq_t[:], in_=gst[:, 1:2], mul=inv_e)
            var_t = singles.tile([128, 1], F32)
            nc.vector.tensor_tensor(out=var_t[:], in0=mean_t[:], in1=mean_t[:],
                                    op=mybir.AluOpType.mult)
            nc.vector.tensor_tensor(out=var_t[:], in0=msq_t[:], in1=var_t[:],
                                    op=mybir.AluOpType.subtract)
            eps_t = singles.tile([128, 1], F32)
            nc.vector.memset(eps_t[:], BN_EPS)
            sd_t = singles.tile([128, 1], F32)
            nc.scalar.activation(out=sd_t[:], in_=var_t[:],
                                 func=mybir.ActivationFunctionType.Sqrt,
                                 bias=eps_t[:], scale=1.0)
            rstd_t = singles.tile([128, 1], F32)
            nc.vector.reciprocal(out=rstd_t[:], in_=sd_t[:])
            s_t = singles.tile([128, 1], F32)
            nc.vector.tensor_tensor(out=s_t[:], in0=rstd_t[:], in1=gb_t[:, 0:1],
                                    op=mybir.AluOpType.mult)
            t_t = singles.tile([128, 1], F32)
            nc.vector.tensor_tensor(out=t_t[:], in0=s_t[:], in1=mean_t[:],
                                    op=mybir.AluOpType.mult)
            nc.vector.tensor_tensor(out=t_t[:], in0=gb_t[:, 1:2], in1=t_t[:],
                                    op=mybir.AluOpType.subtract)

            # ---- phase 2: affine + store (feature-major, call-sized tiles) --
            for cg in range(cfg.H * cfg.C):
                e0 = cg * cfg.NCALL
                vb = min(max(cfg.HV - (cg % cfg.C) * cfg.NCALL, 0), cfg.NCALL)
                if vb == 0:
                    continue
                u = outp.tile([128, cfg.NCALL], BF16, tag="u")
                nc.vector.tensor_scalar(
                    out=u[:, :vb], in0=h3res[:, e0:e0 + vb],
                    scalar1=s_t[:], scalar2=t_t[:],
                    op0=mybir.AluOpType.mult, op1=mybir.AluOpType.add)
                nc.sync.dma_start(out=out[:, e0:e0 + vb], in_=u[:, :vb])
    nc.compile()
    return nc


def get_program(cfg):
    key = (cfg.ES, cfg.H, cfg.C, cfg.B, cfg.TN, cfg.E_TOTAL,
           cfg.skip_collective, cfg.skip_gather, cfg.gq, cfg.gbufs, cfg.banded)
    if key not in _PROG_CACHE:
        _PROG_CACHE[key] = build_program(cfg)
    return _PROG_CACHE[key]


def _wrap_idx(flat, cfg):
    """int16 flat idxs [n] -> wrapped [128, n/16] layout (i at [i%16, i//16], x8)."""
    w = flat.reshape(-1, 16).T.astype(np.int16)      # [16, n/16]
    return np.tile(w, (8, 1))                        # [128, n/16]


def host_prep(node_feats, edge_feats, src, dst, W1, b1, W2, b2, W3, b3, gamma, beta,
              cfg=None):
    cfg = cfg or CFG
    nfb = np.asarray(node_feats, np.float32).astype(ml_dtypes.bfloat16)
    efb = np.asarray(edge_feats, np.float32).astype(ml_dtypes.bfloat16)
    src = np.asarray(src)
    dst = np.asarray(dst)
    W1 = np.asarray(W1, np.float32)

    w1e = W1[:ED].astype(ml_dtypes.bfloat16)
    w1m = W1[ED:].astype(ml_dtypes.bfloat16)
    w2b = np.asarray(W2, np.float32).astype(ml_dtypes.bfloat16)
    w3b = np.asarray(W3, np.float32).astype(ml_dtypes.bfloat16)
    bias = np.stack([np.asarray(b1, np.float32),
                     np.asarray(b2, np.float32),
                     np.asarray(b3, np.float32)], axis=1)          # [128, 3]
    gb = np.stack([np.asarray(gamma, np.float32),
                   np.asarray(beta, np.float32)], axis=1)          # [128, 2]

    if cfg.banded:
        order = np.argsort(src, kind="stable")
        global _LAST_ORDER
        _LAST_ORDER = order
        # P = nf @ W1m: node-level message precompute (bf16-rounded inputs)
        P = (nfb.astype(np.float32) @ W1[ED:]).astype(ml_dtypes.bfloat16)
        src = src[order]
        dst = dst[order]
        efb = efb[order]

    in_maps = []
    for c in range(NCORES):
        base = c * cfg.ES
        tabs, sws = [], []
        binfo = []
        for h in range(cfg.H):
            lo = base + h * cfg.HV
            s_h = src[lo:lo + cfg.HV]
            d_h = dst[lo:lo + cfg.HV]
            if cfg.banded:
                u = np.unique(d_h)
            else:
                u = np.unique(np.concatenate([s_h, d_h]))
            assert len(u) <= cfg.TN, f"local table overflow: {len(u)} > {cfg.TN}"
            assert len(u) <= 32768, "int16 index overflow"
            tab = np.zeros((cfg.TN, ND), ml_dtypes.bfloat16)
            tab[:len(u)] = nfb[u]
            tabs.append(tab)
            d16 = np.searchsorted(u, d_h).astype(np.int16)
            pad = cfg.HP - cfg.HV
            if pad:
                d16 = np.concatenate([d16, np.zeros(pad, np.int16)])
            if cfg.banded:
                s_pad = np.concatenate([s_h, np.full(pad, s_h[-1], np.int32)]) if pad                     else s_h.astype(np.int32)
                binfo.append(s_pad)
                for cl in range(cfg.C):
                    sws.append(_wrap_idx(d16[cl * cfg.NCALL:(cl + 1) * cfg.NCALL], cfg))
            else:
                s16 = np.searchsorted(u, s_h).astype(np.int16)
                if pad:
                    s16 = np.concatenate([s16, np.zeros(pad, np.int16)])
                for cl in range(cfg.C):
                    sws.append(_wrap_idx(s16[cl * cfg.NCALL:(cl + 1) * cfg.NCALL], cfg))
                    sws.append(_wrap_idx(d16[cl * cfg.NCALL:(cl + 1) * cfg.NCALL], cfg))
        srcw = np.concatenate(sws, axis=1)
        # edge feats, transposed + per-half padding
        eftc = np.zeros((ED, cfg.EP), ml_dtypes.bfloat16)
        for h in range(cfg.H):
            lo = base + h * cfg.HV
            eftc[:, h * cfg.HP:h * cfg.HP + cfg.HV] = efb[lo:lo + cfg.HV].T
        im = {"srcw": srcw, "eft": eftc,
              "w1e": w1e, "w1m": w1m, "w2": w2b, "w3": w3b,
              "bias": bias, "gb": gb}
        if cfg.banded:
            s_all = np.concatenate(binfo)                 # [EP] padded sorted src
            nbph = cfg.C * cfg.B
            btabc = np.zeros((cfg.NB * 128, ND), ml_dtypes.bfloat16)
            offc = np.zeros((1, cfg.EP), ml_dtypes.bfloat16)
            for b in range(cfg.NB):
                blk = s_all[b * cfg.BLK:(b + 1) * cfg.BLK]
                w0 = int(blk[0])
                assert int(blk.max()) - w0 < 128, f"block {b} span too wide"
                hi = min(w0 + 128, V)
                btabc[b * 128:b * 128 + (hi - w0)] = P[w0:hi]
                offc[0, b * cfg.BLK:(b + 1) * cfg.BLK] = (blk - w0).astype(
                    ml_dtypes.bfloat16)
            im["btab"] = btabc
            im["offr"] = offc
            im["iot"] = np.arange(128, dtype=np.float32).reshape(128, 1)
            im["ones1"] = np.ones((1, 128), ml_dtypes.bfloat16)
        for h in range(cfg.H):
            im[f"tab{h}"] = tabs[h]
        in_maps.append(im)
    return in_maps


_LAST_ORDER = None


def assemble_output(results, cfg=None):
    cfg = cfg or CFG
    out = np.empty((NCORES * cfg.ES, OUT), np.float32)
    for c in range(NCORES):
        oc = np.asarray(results[c]["out"]).astype(np.float32)   # [OUT, EP]
        for h in range(cfg.H):
            lo = c * cfg.ES + h * cfg.HV
            out[lo:lo + cfg.HV] = oc[:, h * cfg.HP:h * cfg.HP + cfg.HV].T
    if cfg.banded:
        unp = np.empty_like(out)
        unp[_LAST_ORDER] = out
        out = unp
    return out


def kernel(**inputs):
    cfg = CFG
    nc = get_program(cfg)
    in_maps = host_prep(**inputs, cfg=cfg)
    res = run_bass_kernel_spmd(nc, in_maps, list(range(NCORES)))
    return assemble_output(res.results, cfg)


# revision 18
# speedup vs baseline: 1.8004x; 1.0731x over previous
"""Trainium2 Bass kernel for nn_CysInteractLayer (GNN message-passing layer).

out = BN(lrelu(lrelu(lrelu([ef | nf[src]+nf[dst]] @ W1 + b1) @ W2 + b2) @ W3 + b3))

Sharding: edges across 8 NeuronCores; node_feats/weights replicated
(as per-half local node tables so dma_gather's int16 indices suffice);
BN batch stats all-reduced across cores on-device.

v2: feature-major bf16 output (host un-transposes; removes on-chip PE
transposes), bn_stats/bn_aggr for BN statistics, DVE affine in phase 2,
optional multi-queue gathers (gq>1) with per-queue warmup.
"""
import numpy as np
import ml_dtypes

import concourse.bass as bass
import concourse.bacc as bacc
import concourse.tile as tile
from concourse import mybir
from concourse.bass_utils import run_bass_kernel_spmd

F32 = mybir.dt.float32
BF16 = mybir.dt.bfloat16
I16 = mybir.dt.int16

# problem constants (hardcoded per harness contract)
V, E = 50000, 400000
ND, ED, OUT = 128, 64, 128
IN = ND + ED
NEG_SLOPE = 0.01
BN_EPS = 1e-5

NCORES = 8
ES = E // NCORES            # 50000 edges per core


class Cfg:
    """Geometry of the per-core kernel; small instances used for sim tests."""

    def __init__(self, es=ES, halves=2, calls_per_half=7, blocks_per_call=7,
                 tn=32768, e_total=E, skip_collective=False, skip_gather=False,
                 gq=1, gbufs=4, banded=True):
        self.banded = banded
        self.skip_collective = skip_collective
        self.skip_gather = skip_gather
        self.gq = gq
        self.gbufs = gbufs
        self.ES = es                        # valid edges per core
        self.H = halves
        self.C = calls_per_half
        self.B = blocks_per_call
        self.BLK = 512
        self.TN = tn                        # local table rows (padded)
        self.E_TOTAL = e_total
        self.HP = self.C * self.B * self.BLK   # padded edges per half
        self.EP = self.H * self.HP             # padded edges per core
        self.HV = es // halves                 # valid edges per half
        assert es % halves == 0
        assert self.HP >= self.HV
        self.NB = self.H * self.C * self.B     # total blocks
        self.NCALL = self.B * self.BLK         # edges per gather call
        self.WCOLS = self.NCALL // 16          # wrapped idx cols per call

    def block_valid(self, b):
        """Valid columns in global block b (rest is padding)."""
        h, lb = divmod(b, self.C * self.B)
        lo = lb * self.BLK
        return int(np.clip(self.HV - lo, 0, self.BLK))


CFG = Cfg()

_PROG_CACHE = {}


def build_program(cfg):
    nc = bacc.Bacc(None, target_bir_lowering=False, num_swdge_queues=4)

    tabs = [nc.declare_dram_parameter(f"tab{h}", [cfg.TN, ND], BF16, isOutput=False)
            for h in range(cfg.H)]
    nidx = (1 if cfg.banded else 2)
    srcw = nc.declare_dram_parameter("srcw", [128, nidx * cfg.H * cfg.C * cfg.WCOLS], I16, isOutput=False)
    if cfg.banded:
        btab = nc.declare_dram_parameter("btab", [cfg.NB * 128, ND], BF16, isOutput=False)
        offr = nc.declare_dram_parameter("offr", [1, cfg.EP], BF16, isOutput=False)
        iot = nc.declare_dram_parameter("iot", [128, 1], F32, isOutput=False)
        ones1 = nc.declare_dram_parameter("ones1", [1, 128], BF16, isOutput=False)
    eft = nc.declare_dram_parameter("eft", [ED, cfg.EP], BF16, isOutput=False)
    w1e = nc.declare_dram_parameter("w1e", [ED, OUT], BF16, isOutput=False)
    w1m = nc.declare_dram_parameter("w1m", [ND, OUT], BF16, isOutput=False)
    w2 = nc.declare_dram_parameter("w2", [OUT, OUT], BF16, isOutput=False)
    w3 = nc.declare_dram_parameter("w3", [OUT, OUT], BF16, isOutput=False)
    bias = nc.declare_dram_parameter("bias", [128, 3], F32, isOutput=False)   # b1|b2|b3
    gb = nc.declare_dram_parameter("gb", [128, 2], F32, isOutput=False)       # gamma|beta
    # feature-major output; host transposes back to [EP, OUT]
    out = nc.declare_dram_parameter("out", [OUT, cfg.EP], BF16, isOutput=True)

    LR = mybir.ActivationFunctionType.Lrelu

    with tile.TileContext(nc) as tc:
        with (
            tc.tile_pool(name="singles", bufs=1) as singles,
            tc.tile_pool(name="hres", bufs=1) as hres,
            tc.tile_pool(name="gat", bufs=cfg.gbufs) as gat,
            tc.tile_pool(name="eftp", bufs=2) as eftp,
            tc.tile_pool(name="work", bufs=3) as work,
            tc.tile_pool(name="outp", bufs=2) as outp,
            tc.tile_pool(name="zps", bufs=4, space="PSUM") as zps,
            tc.tile_pool(name="dram", bufs=1, space="DRAM") as dram,
        ):
            # ---- static state (srcw first: gathers depend on it) -------------
            srcw_t = singles.tile([128, nidx * cfg.H * cfg.C * cfg.WCOLS], I16)
            hw_cols = nidx * cfg.C * cfg.WCOLS
            for h in range(cfg.H):
                nc.sync.dma_start(out=srcw_t[:, h * hw_cols:(h + 1) * hw_cols],
                                  in_=srcw[:, h * hw_cols:(h + 1) * hw_cols])
            if cfg.banded:
                iot_t = singles.tile([128, 1], F32)
                nc.sync.dma_start(out=iot_t[:], in_=iot[:, :])
                ones1_t = singles.tile([1, 128], BF16)
                nc.sync.dma_start(out=ones1_t[:], in_=ones1[:, :])
            w1e_t = singles.tile([ED, OUT], BF16)
            nc.sync.dma_start(out=w1e_t[:], in_=w1e[:, :])
            w1m_t = singles.tile([ND, OUT], BF16)
            nc.sync.dma_start(out=w1m_t[:], in_=w1m[:, :])
            w2_t = singles.tile([OUT, OUT], BF16)
            nc.sync.dma_start(out=w2_t[:], in_=w2[:, :])
            w3_t = singles.tile([OUT, OUT], BF16)
            nc.sync.dma_start(out=w3_t[:], in_=w3[:, :])
            bias_t = singles.tile([128, 3], F32)
            nc.sync.dma_start(out=bias_t[:], in_=bias[:, :])
            gb_t = singles.tile([128, 2], F32)
            nc.sync.dma_start(out=gb_t[:], in_=gb[:, :])

            h3res = hres.tile([128, cfg.EP], BF16)
            stat_stripe = singles.tile([128, cfg.NB, 6], F32)

            # ---- warmup: one tiny gather per queue used (first-use race) -----
            if cfg.gq > 1 and not cfg.skip_gather:
                wsc = singles.tile([128, 1], F32)
                for q in range(cfg.gq):
                    wg = gat.tile([128, 1, 128], BF16, tag="gsd")
                    nc.gpsimd.dma_gather(
                        out_ap=wg[:], in_ap=tabs[0][:, :],
                        idxs_ap=srcw_t[:, 0:8],
                        num_idxs=128, num_idxs_reg=128,
                        elem_size=ND, transpose=True, single_packet=False,
                        queue_num=q,
                        )
                    nc.vector.tensor_reduce(
                        out=wsc[:], in_=wg[:, 0, :],
                        axis=mybir.AxisListType.X, op=mybir.AluOpType.add)

            # ---- phase 1: gather + MLP + stats -------------------------------
            for h in range(cfg.H):
                for c in range(cfg.C):
                    wofs = (h * cfg.C + c) * cfg.WCOLS
                    gsd = gat.tile([128, 1, nidx * cfg.NCALL], BF16, tag="gsd")
                    if cfg.skip_gather:
                        nc.vector.memset(gsd[:], 0.25)
                    else:
                        nc.gpsimd.dma_gather(
                            out_ap=gsd[:], in_ap=tabs[h][:, :],
                            idxs_ap=srcw_t[:, nidx * wofs:nidx * (wofs + cfg.WCOLS)],
                            num_idxs=nidx * cfg.NCALL, num_idxs_reg=nidx * cfg.NCALL,
                            elem_size=ND, transpose=True, single_packet=False,
                            queue_num=(h * cfg.C + c) % cfg.gq,
                        )
                    if cfg.banded:
                        gd = gsd[:, :, :cfg.NCALL]
                    else:
                        gs = gsd[:, :, :cfg.NCALL]
                        gd = gsd[:, :, cfg.NCALL:]
                    call_e0 = (h * cfg.C + c) * cfg.NCALL
                    eft_t = eftp.tile([ED, cfg.NCALL], BF16, tag="eft")
                    nc.sync.dma_start(out=eft_t[:], in_=eft[:, call_e0:call_e0 + cfg.NCALL])
                    if cfg.banded:
                        offc_t = eftp.tile([1, cfg.NCALL], BF16, tag="offc")
                        nc.sync.dma_start(out=offc_t[:], in_=offr[0:1, call_e0:call_e0 + cfg.NCALL])

                    for b in range(cfg.B):
                        gb_idx = (h * cfg.C + c) * cfg.B + b
                        vb = cfg.block_valid(gb_idx)
                        if vb == 0:
                            continue
                        co = b * cfg.BLK          # col offset within call
                        e0 = call_e0 + co         # global (padded) edge offset
                        zp = zps.tile([128, cfg.BLK], F32, tag="z")
                        if cfg.banded:
                            zoff = zps.tile([128, cfg.BLK], F32, tag="zoff")
                            nc.tensor.matmul(zoff[:, :], lhsT=ones1_t[:],
                                             rhs=offc_t[0:1, co:co + cfg.BLK],
                                             start=True, stop=True)
                            oh = work.tile([128, cfg.BLK], BF16, tag="oh")
                            nc.vector.tensor_scalar(
                                out=oh[:], in0=zoff[:], scalar1=iot_t[:], scalar2=None,
                                op0=mybir.AluOpType.is_equal)
                            bt = eftp.tile([128, ND], BF16, tag="bt")
                            nc.sync.dma_start(
                                out=bt[:], in_=btab[gb_idx * 128:(gb_idx + 1) * 128, :])
                            nc.tensor.matmul(zp[:, :], lhsT=bt[:], rhs=oh[:],
                                             start=True, stop=False)
                        else:
                            nc.tensor.matmul(zp[:, :], lhsT=w1m_t[:], rhs=gs[:, 0, co:co + cfg.BLK],
                                             start=True, stop=False)
                        nc.tensor.matmul(zp[:, :], lhsT=w1e_t[:], rhs=eft_t[:, co:co + cfg.BLK],
                                         start=False, stop=False)
                        nc.tensor.matmul(zp[:, :], lhsT=w1m_t[:], rhs=gd[:, 0, co:co + cfg.BLK],
                                         start=False, stop=True)
                        h1 = work.tile([128, cfg.BLK], BF16, tag="h1")
                        nc.scalar.activation(out=h1[:], in_=zp[:], func=LR,
                                             bias=bias_t[:, 0:1], scale=1.0, alpha=NEG_SLOPE)
                        zp2 = zps.tile([128, cfg.BLK], F32, tag="z")
                        nc.tensor.matmul(zp2[:, :], lhsT=w2_t[:], rhs=h1[:], start=True, stop=True)
                        h2 = work.tile([128, cfg.BLK], BF16, tag="h2")
                        nc.scalar.activation(out=h2[:], in_=zp2[:], func=LR,
                                             bias=bias_t[:, 1:2], scale=1.0, alpha=NEG_SLOPE)
                        zp3 = zps.tile([128, cfg.BLK], F32, tag="z")
                        nc.tensor.matmul(zp3[:, :], lhsT=w3_t[:], rhs=h2[:], start=True, stop=True)
                        h3 = h3res[:, e0:e0 + cfg.BLK]
                        nc.scalar.activation(out=h3[:, :vb], in_=zp3[:, :vb], func=LR,
                                             bias=bias_t[:, 2:3], scale=1.0, alpha=NEG_SLOPE)
                        nc.vector.bn_stats(out=stat_stripe[:, gb_idx, :],
                                           in_=h3[:, :vb])

            # ---- stats + allreduce ------------------------------------------
            hv_blocks = (cfg.HV + cfg.BLK - 1) // cfg.BLK
            mv = singles.tile([128, 2], F32)
            nvalid = sum(1 for b in range(cfg.NB) if cfg.block_valid(b) > 0)
            assert nvalid == cfg.NB, "stat stripe has gaps; aggregate per half"
            nc.vector.bn_aggr(out=mv[:], in_=stat_stripe[:])

            st2 = singles.tile([128, 2], F32)
            nc.scalar.mul(out=st2[:, 0:1], in_=mv[:, 0:1], mul=float(cfg.ES))
            msq = singles.tile([128, 1], F32)
            nc.vector.tensor_tensor(out=msq[:], in0=mv[:, 0:1], in1=mv[:, 0:1],
                                    op=mybir.AluOpType.mult)
            nc.vector.tensor_tensor(out=msq[:], in0=mv[:, 1:2], in1=msq[:],
                                    op=mybir.AluOpType.add)
            nc.scalar.mul(out=st2[:, 1:2], in_=msq[:], mul=float(cfg.ES))

            cc_in = dram.tile([128, 2], F32)
            cc_out = dram.tile([128, 2], F32)
            nc.gpsimd.dma_start(out=cc_in[:], in_=st2[:])
            if cfg.skip_collective:
                nc.gpsimd.dma_start(out=cc_out[:], in_=cc_in[:])
            else:
                nc.gpsimd.collective_compute(
                    "AllReduce", mybir.AluOpType.add,
                    replica_groups=[list(range(NCORES))],
                    ins=[cc_in.opt()], outs=[cc_out.opt()],
                )
            gst = singles.tile([128, 2], F32)
            nc.gpsimd.dma_start(out=gst[:], in_=cc_out[:])

            inv_e = 1.0 / cfg.E_TOTAL
            mean_t = singles.tile([128, 1], F32)
            nc.scalar.mul(out=mean_t[:], in_=gst[:, 0:1], mul=inv_e)
            msq_t = singles.tile([128, 1], F32)
            nc.scalar.mul(out=msq_t[:], in_=gst[:, 1:2], mul=inv_e)
            var_t = singles.tile([128, 1], F32)
            nc.vector.tensor_tensor(out=var_t[:], in0=mean_t[:], in1=mean_t[:],
                                    op=mybir.AluOpType.mult)
            nc.vector.tensor_tensor(out=var_t[:], in0=msq_t[:], in1=var_t[:],
                                    op=mybir.AluOpType.subtract)
            eps_t = singles.tile([128, 1], F32)
            nc.vector.memset(eps_t[:], BN_EPS)
            sd_t = singles.tile([128, 1], F32)
            nc.scalar.activation(out=sd_t[:], in_=var_t[:],
                                 func=mybir.ActivationFunctionType.Sqrt,
                                 bias=eps_t[:], scale=1.0)
            rstd_t = singles.tile([128, 1], F32)
            nc.vector.reciprocal(out=rstd_t[:], in_=sd_t[:])
            s_t = singles.tile([128, 1], F32)
            nc.vector.tensor_tensor(out=s_t[:], in0=rstd_t[:], in1=gb_t[:, 0:1],
                                    op=mybir.AluOpType.mult)
            t_t = singles.tile([128, 1], F32)
            nc.vector.tensor_tensor(out=t_t[:], in0=s_t[:], in1=mean_t[:],
                                    op=mybir.AluOpType.mult)
            nc.vector.tensor_tensor(out=t_t[:], in0=gb_t[:, 1:2], in1=t_t[:],
                                    op=mybir.AluOpType.subtract)

            # ---- phase 2: affine + store (feature-major, call-sized tiles) --
            for cg in range(cfg.H * cfg.C):
                e0 = cg * cfg.NCALL
                vb = min(max(cfg.HV - (cg % cfg.C) * cfg.NCALL, 0), cfg.NCALL)
                if vb == 0:
                    continue
                u = outp.tile([128, cfg.NCALL], BF16, tag="u")
                nc.vector.tensor_scalar(
                    out=u[:, :vb], in0=h3res[:, e0:e0 + vb],
                    scalar1=s_t[:], scalar2=t_t[:],
                    op0=mybir.AluOpType.mult, op1=mybir.AluOpType.add)
                nc.sync.dma_start(out=out[:, e0:e0 + vb], in_=u[:, :vb])
    nc.compile()
    return nc


def get_program(cfg):
    key = (cfg.ES, cfg.H, cfg.C, cfg.B, cfg.TN, cfg.E_TOTAL,
           cfg.skip_collective, cfg.skip_gather, cfg.gq, cfg.gbufs, cfg.banded)
    if key not in _PROG_CACHE:
        _PROG_CACHE[key] = build_program(cfg)
    return _PROG_CACHE[key]


def _wrap_idx(flat, cfg):
    """int16 flat idxs [n] -> wrapped [128, n/16] layout (i at [i%16, i//16], x8)."""
    w = flat.reshape(-1, 16).T.astype(np.int16)      # [16, n/16]
    return np.tile(w, (8, 1))                        # [128, n/16]


def host_prep(node_feats, edge_feats, src, dst, W1, b1, W2, b2, W3, b3, gamma, beta,
              cfg=None):
    cfg = cfg or CFG
    nfb = np.asarray(node_feats, np.float32).astype(ml_dtypes.bfloat16)
    efb = np.asarray(edge_feats, np.float32).astype(ml_dtypes.bfloat16)
    src = np.asarray(src)
    dst = np.asarray(dst)
    W1 = np.asarray(W1, np.float32)

    w1e = W1[:ED].astype(ml_dtypes.bfloat16)
    w1m = W1[ED:].astype(ml_dtypes.bfloat16)
    w2b = np.asarray(W2, np.float32).astype(ml_dtypes.bfloat16)
    w3b = np.asarray(W3, np.float32).astype(ml_dtypes.bfloat16)
    bias = np.stack([np.asarray(b1, np.float32),
                     np.asarray(b2, np.float32),
                     np.asarray(b3, np.float32)], axis=1)          # [128, 3]
    gb = np.stack([np.asarray(gamma, np.float32),
                   np.asarray(beta, np.float32)], axis=1)          # [128, 2]

    if cfg.banded:
        order = np.argsort(src, kind="stable")
        global _LAST_ORDER
        _LAST_ORDER = order
        # P = nf @ W1m: node-level message precompute (bf16-rounded inputs)
        P = (nfb.astype(np.float32) @ W1[ED:]).astype(ml_dtypes.bfloat16)
        src = src[order]
        dst = dst[order]
        efb = efb[order]

    in_maps = []
    for c in range(NCORES):
        base = c * cfg.ES
        tabs, sws = [], []
        binfo = []
        for h in range(cfg.H):
            lo = base + h * cfg.HV
            s_h = src[lo:lo + cfg.HV]
            d_h = dst[lo:lo + cfg.HV]
            if cfg.banded:
                u = np.unique(d_h)
            else:
                u = np.unique(np.concatenate([s_h, d_h]))
            assert len(u) <= cfg.TN, f"local table overflow: {len(u)} > {cfg.TN}"
            assert len(u) <= 32768, "int16 index overflow"
            tab = np.zeros((cfg.TN, ND), ml_dtypes.bfloat16)
            tab[:len(u)] = nfb[u]
            tabs.append(tab)
            d16 = np.searchsorted(u, d_h).astype(np.int16)
            pad = cfg.HP - cfg.HV
            if pad:
                d16 = np.concatenate([d16, np.zeros(pad, np.int16)])
            if cfg.banded:
                s_pad = np.concatenate([s_h, np.full(pad, s_h[-1], np.int32)]) if pad                     else s_h.astype(np.int32)
                binfo.append(s_pad)
                for cl in range(cfg.C):
                    sws.append(_wrap_idx(d16[cl * cfg.NCALL:(cl + 1) * cfg.NCALL], cfg))
            else:
                s16 = np.searchsorted(u, s_h).astype(np.int16)
                if pad:
                    s16 = np.concatenate([s16, np.zeros(pad, np.int16)])
                for cl in range(cfg.C):
                    sws.append(_wrap_idx(s16[cl * cfg.NCALL:(cl + 1) * cfg.NCALL], cfg))
                    sws.append(_wrap_idx(d16[cl * cfg.NCALL:(cl + 1) * cfg.NCALL], cfg))
        srcw = np.concatenate(sws, axis=1)
        # edge feats, transposed + per-half padding
        eftc = np.zeros((ED, cfg.EP), ml_dtypes.bfloat16)
        for h in range(cfg.H):
            lo = base + h * cfg.HV
            eftc[:, h * cfg.HP:h * cfg.HP + cfg.HV] = efb[lo:lo + cfg.HV].T
        im = {"srcw": srcw, "eft": eftc,
              "w1e": w1e, "w1m": w1m, "w2": w2b, "w3": w3b,
              "bias": bias, "gb": gb}
        if cfg.banded:
            s_all = np.concatenate(binfo)                 # [EP] padded sorted src
            nbph = cfg.C * cfg.B
            btabc = np.zeros((cfg.NB * 128, ND), ml_dtypes.bfloat16)
            offc = np.zeros((1, cfg.EP), ml_dtypes.bfloat16)
            for b in range(cfg.NB):
                blk = s_all[b * cfg.BLK:(b + 1) * cfg.BLK]
                w0 = int(blk[0])
                assert int(blk.max()) - w0 < 128, f"block {b} span too wide"
                hi = min(w0 + 128, V)
                btabc[b * 128:b * 128 + (hi - w0)] = P[w0:hi]
                offc[0, b * cfg.BLK:(b + 1) * cfg.BLK] = (blk - w0).astype(
                    ml_dtypes.bfloat16)
            im["btab"] = btabc
            im["offr"] = offc
            im["iot"] = np.arange(128, dtype=np.float32).reshape(128, 1)
            im["ones1"] = np.ones((1, 128), ml_dtypes.bfloat16)
        for h in range(cfg.H):
            im[f"tab{h}"] = tabs[h]
        in_maps.append(im)
    return in_maps


_LAST_ORDER = None


def assemble_output(results, cfg=None):
    cfg = cfg or CFG
    out = np.empty((NCORES * cfg.ES, OUT), np.float32)
    for c in range(NCORES):
        oc = np.asarray(results[c]["out"]).astype(np.float32)   # [OUT, EP]
        for h in range(cfg.H):
            lo = c * cfg.ES + h * cfg.HV
            out[lo:lo + cfg.HV] = oc[:, h * cfg.HP:h * cfg.HP + cfg.HV].T
    if cfg.banded:
        unp = np.empty_like(out)
        unp[_LAST_ORDER] = out
        out = unp
    return out


def kernel(**inputs):
    cfg = CFG
    nc = get_program(cfg)
    in_maps = host_prep(**inputs, cfg=cfg)
    res = run_bass_kernel_spmd(nc, in_maps, list(range(NCORES)))
    return assemble_output(res.results, cfg)


# revision 19
# speedup vs baseline: 1.8072x; 1.0038x over previous
"""Trainium2 Bass kernel for nn_CysInteractLayer (GNN message-passing layer).

out = BN(lrelu(lrelu(lrelu([ef | nf[src]+nf[dst]] @ W1 + b1) @ W2 + b2) @ W3 + b3))

Sharding: edges across 8 NeuronCores; node_feats/weights replicated
(as per-half local node tables so dma_gather's int16 indices suffice);
BN batch stats all-reduced across cores on-device.

v2: feature-major bf16 output (host un-transposes; removes on-chip PE
transposes), bn_stats/bn_aggr for BN statistics, DVE affine in phase 2,
optional multi-queue gathers (gq>1) with per-queue warmup.
"""
import numpy as np
import ml_dtypes

import concourse.bass as bass
import concourse.bacc as bacc
import concourse.tile as tile
from concourse import mybir
from concourse.bass_utils import run_bass_kernel_spmd

F32 = mybir.dt.float32
BF16 = mybir.dt.bfloat16
I16 = mybir.dt.int16

# problem constants (hardcoded per harness contract)
V, E = 50000, 400000
ND, ED, OUT = 128, 64, 128
IN = ND + ED
NEG_SLOPE = 0.01
BN_EPS = 1e-5

NCORES = 8
ES = E // NCORES            # 50000 edges per core


class Cfg:
    """Geometry of the per-core kernel; small instances used for sim tests."""

    def __init__(self, es=ES, halves=2, calls_per_half=7, blocks_per_call=7,
                 tn=32768, e_total=E, skip_collective=False, skip_gather=False,
                 gq=1, gbufs=4, banded=True):
        self.banded = banded
        self.skip_collective = skip_collective
        self.skip_gather = skip_gather
        self.gq = gq
        self.gbufs = gbufs
        self.ES = es                        # valid edges per core
        self.H = halves
        self.C = calls_per_half
        self.B = blocks_per_call
        self.BLK = 512
        self.TN = tn                        # local table rows (padded)
        self.E_TOTAL = e_total
        self.HP = self.C * self.B * self.BLK   # padded edges per half
        self.EP = self.H * self.HP             # padded edges per core
        self.HV = es // halves                 # valid edges per half
        assert es % halves == 0
        assert self.HP >= self.HV
        self.NB = self.H * self.C * self.B     # total blocks
        self.NCALL = self.B * self.BLK         # edges per gather call
        self.WCOLS = self.NCALL // 16          # wrapped idx cols per call

    def block_valid(self, b):
        """Valid columns in global block b (rest is padding)."""
        h, lb = divmod(b, self.C * self.B)
        lo = lb * self.BLK
        return int(np.clip(self.HV - lo, 0, self.BLK))


CFG = Cfg()

_PROG_CACHE = {}


def build_program(cfg):
    nc = bacc.Bacc(None, target_bir_lowering=False, num_swdge_queues=4)

    tabs = [nc.declare_dram_parameter(f"tab{h}", [cfg.TN, ND], BF16, isOutput=False)
            for h in range(cfg.H)]
    nidx = (1 if cfg.banded else 2)
    srcw = nc.declare_dram_parameter("srcw", [128, nidx * cfg.H * cfg.C * cfg.WCOLS], I16, isOutput=False)
    if cfg.banded:
        btab = nc.declare_dram_parameter("btab", [cfg.NB * 128, ND], BF16, isOutput=False)
        offr = nc.declare_dram_parameter("offr", [1, cfg.EP], BF16, isOutput=False)
        iot = nc.declare_dram_parameter("iot", [128, 1], F32, isOutput=False)
        ones1 = nc.declare_dram_parameter("ones1", [1, 128], BF16, isOutput=False)
    eft = nc.declare_dram_parameter("eft", [ED, cfg.EP], BF16, isOutput=False)
    w1e = nc.declare_dram_parameter("w1e", [ED, OUT], BF16, isOutput=False)
    w1m = nc.declare_dram_parameter("w1m", [ND, OUT], BF16, isOutput=False)
    w2 = nc.declare_dram_parameter("w2", [OUT, OUT], BF16, isOutput=False)
    w3 = nc.declare_dram_parameter("w3", [OUT, OUT], BF16, isOutput=False)
    bias = nc.declare_dram_parameter("bias", [128, 3], F32, isOutput=False)   # b1|b2|b3
    gb = nc.declare_dram_parameter("gb", [128, 2], F32, isOutput=False)       # gamma|beta
    # feature-major output; host transposes back to [EP, OUT]
    out = nc.declare_dram_parameter("out", [OUT, cfg.EP], BF16, isOutput=True)

    LR = mybir.ActivationFunctionType.Lrelu

    with tile.TileContext(nc) as tc:
        with (
            tc.tile_pool(name="singles", bufs=1) as singles,
            tc.tile_pool(name="hres", bufs=1) as hres,
            tc.tile_pool(name="gat", bufs=cfg.gbufs) as gat,
            tc.tile_pool(name="eftp", bufs=2) as eftp,
            tc.tile_pool(name="work", bufs=3) as work,
            tc.tile_pool(name="outp", bufs=4) as outp,
            tc.tile_pool(name="zps", bufs=4, space="PSUM") as zps,
            tc.tile_pool(name="dram", bufs=1, space="DRAM") as dram,
        ):
            # ---- static state (srcw first: gathers depend on it) -------------
            srcw_t = singles.tile([128, nidx * cfg.H * cfg.C * cfg.WCOLS], I16)
            hw_cols = nidx * cfg.C * cfg.WCOLS
            for h in range(cfg.H):
                nc.sync.dma_start(out=srcw_t[:, h * hw_cols:(h + 1) * hw_cols],
                                  in_=srcw[:, h * hw_cols:(h + 1) * hw_cols])
            if cfg.banded:
                iot_t = singles.tile([128, 1], F32)
                nc.sync.dma_start(out=iot_t[:], in_=iot[:, :])
                ones1_t = singles.tile([1, 128], BF16)
                nc.sync.dma_start(out=ones1_t[:], in_=ones1[:, :])
            w1e_t = singles.tile([ED, OUT], BF16)
            nc.sync.dma_start(out=w1e_t[:], in_=w1e[:, :])
            w1m_t = singles.tile([ND, OUT], BF16)
            nc.sync.dma_start(out=w1m_t[:], in_=w1m[:, :])
            w2_t = singles.tile([OUT, OUT], BF16)
            nc.sync.dma_start(out=w2_t[:], in_=w2[:, :])
            w3_t = singles.tile([OUT, OUT], BF16)
            nc.sync.dma_start(out=w3_t[:], in_=w3[:, :])
            bias_t = singles.tile([128, 3], F32)
            nc.sync.dma_start(out=bias_t[:], in_=bias[:, :])
            gb_t = singles.tile([128, 2], F32)
            nc.sync.dma_start(out=gb_t[:], in_=gb[:, :])

            h3res = hres.tile([128, cfg.EP], BF16)
            stat_stripe = singles.tile([128, cfg.NB, 6], F32)

            # ---- warmup: one tiny gather per queue used (first-use race) -----
            if cfg.gq > 1 and not cfg.skip_gather:
                wsc = singles.tile([128, 1], F32)
                for q in range(cfg.gq):
                    wg = gat.tile([128, 1, 128], BF16, tag="gsd")
                    nc.gpsimd.dma_gather(
                        out_ap=wg[:], in_ap=tabs[0][:, :],
                        idxs_ap=srcw_t[:, 0:8],
                        num_idxs=128, num_idxs_reg=128,
                        elem_size=ND, transpose=True, single_packet=False,
                        queue_num=q,
                        )
                    nc.vector.tensor_reduce(
                        out=wsc[:], in_=wg[:, 0, :],
                        axis=mybir.AxisListType.X, op=mybir.AluOpType.add)

            # ---- phase 1: gather + MLP + stats -------------------------------
            for h in range(cfg.H):
                for c in range(cfg.C):
                    wofs = (h * cfg.C + c) * cfg.WCOLS
                    gsd = gat.tile([128, 1, nidx * cfg.NCALL], BF16, tag="gsd")
                    last_call = cfg.banded and (h == cfg.H - 1) and (c == cfg.C - 1)
                    if cfg.skip_gather:
                        nc.vector.memset(gsd[:], 0.25)
                    elif last_call:
                        # per-block gathers: lets the tail MLP pipeline instead
                        # of waiting for one monolithic gather
                        bw = cfg.BLK // 16
                        for bb in range(cfg.B):
                            nc.gpsimd.dma_gather(
                                out_ap=gsd[:, :, bb * cfg.BLK:(bb + 1) * cfg.BLK],
                                in_ap=tabs[h][:, :],
                                idxs_ap=srcw_t[:, wofs + bb * bw:wofs + (bb + 1) * bw],
                                num_idxs=cfg.BLK, num_idxs_reg=cfg.BLK,
                                elem_size=ND, transpose=True, single_packet=False,
                            )
                    else:
                        nc.gpsimd.dma_gather(
                            out_ap=gsd[:], in_ap=tabs[h][:, :],
                            idxs_ap=srcw_t[:, nidx * wofs:nidx * (wofs + cfg.WCOLS)],
                            num_idxs=nidx * cfg.NCALL, num_idxs_reg=nidx * cfg.NCALL,
                            elem_size=ND, transpose=True, single_packet=False,
                            queue_num=(h * cfg.C + c) % cfg.gq,
                        )
                    if cfg.banded:
                        gd = gsd[:, :, :cfg.NCALL]
                    else:
                        gs = gsd[:, :, :cfg.NCALL]
                        gd = gsd[:, :, cfg.NCALL:]
                    call_e0 = (h * cfg.C + c) * cfg.NCALL
                    eft_t = eftp.tile([ED, cfg.NCALL], BF16, tag="eft")
                    nc.sync.dma_start(out=eft_t[:], in_=eft[:, call_e0:call_e0 + cfg.NCALL])
                    if cfg.banded:
                        offc_t = eftp.tile([1, cfg.NCALL], BF16, tag="offc")
                        nc.sync.dma_start(out=offc_t[:], in_=offr[0:1, call_e0:call_e0 + cfg.NCALL])

                    for b in range(cfg.B):
                        gb_idx = (h * cfg.C + c) * cfg.B + b
                        vb = cfg.block_valid(gb_idx)
                        if vb == 0:
                            continue
                        co = b * cfg.BLK          # col offset within call
                        e0 = call_e0 + co         # global (padded) edge offset
                        zp = zps.tile([128, cfg.BLK], F32, tag="z")
                        if cfg.banded:
                            zoff = zps.tile([128, cfg.BLK], F32, tag="zoff")
                            nc.tensor.matmul(zoff[:, :], lhsT=ones1_t[:],
                                             rhs=offc_t[0:1, co:co + cfg.BLK],
                                             start=True, stop=True)
                            oh = work.tile([128, cfg.BLK], BF16, tag="oh")
                            nc.vector.tensor_scalar(
                                out=oh[:], in0=zoff[:], scalar1=iot_t[:], scalar2=None,
                                op0=mybir.AluOpType.is_equal)
                            bt = eftp.tile([128, ND], BF16, tag="bt")
                            nc.sync.dma_start(
                                out=bt[:], in_=btab[gb_idx * 128:(gb_idx + 1) * 128, :])
                            nc.tensor.matmul(zp[:, :], lhsT=bt[:], rhs=oh[:],
                                             start=True, stop=False)
                        else:
                            nc.tensor.matmul(zp[:, :], lhsT=w1m_t[:], rhs=gs[:, 0, co:co + cfg.BLK],
                                             start=True, stop=False)
                        nc.tensor.matmul(zp[:, :], lhsT=w1e_t[:], rhs=eft_t[:, co:co + cfg.BLK],
                                         start=False, stop=False)
                        nc.tensor.matmul(zp[:, :], lhsT=w1m_t[:], rhs=gd[:, 0, co:co + cfg.BLK],
                                         start=False, stop=True)
                        h1 = work.tile([128, cfg.BLK], BF16, tag="h1")
                        nc.scalar.activation(out=h1[:], in_=zp[:], func=LR,
                                             bias=bias_t[:, 0:1], scale=1.0, alpha=NEG_SLOPE)
                        zp2 = zps.tile([128, cfg.BLK], F32, tag="z")
                        nc.tensor.matmul(zp2[:, :], lhsT=w2_t[:], rhs=h1[:], start=True, stop=True)
                        h2 = work.tile([128, cfg.BLK], BF16, tag="h2")
                        nc.scalar.activation(out=h2[:], in_=zp2[:], func=LR,
                                             bias=bias_t[:, 1:2], scale=1.0, alpha=NEG_SLOPE)
                        zp3 = zps.tile([128, cfg.BLK], F32, tag="z")
                        nc.tensor.matmul(zp3[:, :], lhsT=w3_t[:], rhs=h2[:], start=True, stop=True)
                        h3 = h3res[:, e0:e0 + cfg.BLK]
                        nc.scalar.activation(out=h3[:, :vb], in_=zp3[:, :vb], func=LR,
                                             bias=bias_t[:, 2:3], scale=1.0, alpha=NEG_SLOPE)
                        nc.vector.bn_stats(out=stat_stripe[:, gb_idx, :],
                                           in_=h3[:, :vb])

            # ---- stats + allreduce ------------------------------------------
            hv_blocks = (cfg.HV + cfg.BLK - 1) // cfg.BLK
            mv = singles.tile([128, 2], F32)
            nvalid = sum(1 for b in range(cfg.NB) if cfg.block_valid(b) > 0)
            assert nvalid == cfg.NB, "stat stripe has gaps; aggregate per half"
            nc.vector.bn_aggr(out=mv[:], in_=stat_stripe[:])

            st2 = singles.tile([128, 2], F32)
            nc.scalar.mul(out=st2[:, 0:1], in_=mv[:, 0:1], mul=float(cfg.ES))
            msq = singles.tile([128, 1], F32)
            nc.vector.tensor_tensor(out=msq[:], in0=mv[:, 0:1], in1=mv[:, 0:1],
                                    op=mybir.AluOpType.mult)
            nc.vector.tensor_tensor(out=msq[:], in0=mv[:, 1:2], in1=msq[:],
                                    op=mybir.AluOpType.add)
            nc.scalar.mul(out=st2[:, 1:2], in_=msq[:], mul=float(cfg.ES))

            cc_in = dram.tile([128, 2], F32)
            cc_out = dram.tile([128, 2], F32)
            nc.sync.dma_start(out=cc_in[:], in_=st2[:])
            if cfg.skip_collective:
                nc.gpsimd.dma_start(out=cc_out[:], in_=cc_in[:])
            else:
                nc.gpsimd.collective_compute(
                    "AllReduce", mybir.AluOpType.add,
                    replica_groups=[list(range(NCORES))],
                    ins=[cc_in.opt()], outs=[cc_out.opt()],
                )
            gst = singles.tile([128, 2], F32)
            nc.sync.dma_start(out=gst[:], in_=cc_out[:])

            inv_e = 1.0 / cfg.E_TOTAL
            mean_t = singles.tile([128, 1], F32)
            nc.scalar.mul(out=mean_t[:], in_=gst[:, 0:1], mul=inv_e)
            msq_t = singles.tile([128, 1], F32)
            nc.scalar.mul(out=msq_t[:], in_=gst[:, 1:2], mul=inv_e)
            var_t = singles.tile([128, 1], F32)
            nc.vector.tensor_tensor(out=var_t[:], in0=mean_t[:], in1=mean_t[:],
                                    op=mybir.AluOpType.mult)
            nc.vector.tensor_tensor(out=var_t[:], in0=msq_t[:], in1=var_t[:],
                                    op=mybir.AluOpType.subtract)
            eps_t = singles.tile([128, 1], F32)
            nc.vector.memset(eps_t[:], BN_EPS)
            sd_t = singles.tile([128, 1], F32)
            nc.scalar.activation(out=sd_t[:], in_=var_t[:],
                                 func=mybir.ActivationFunctionType.Sqrt,
                                 bias=eps_t[:], scale=1.0)
            rstd_t = singles.tile([128, 1], F32)
            nc.vector.reciprocal(out=rstd_t[:], in_=sd_t[:])
            s_t = singles.tile([128, 1], F32)
            nc.vector.tensor_tensor(out=s_t[:], in0=rstd_t[:], in1=gb_t[:, 0:1],
                                    op=mybir.AluOpType.mult)
            t_t = singles.tile([128, 1], F32)
            nc.vector.tensor_tensor(out=t_t[:], in0=s_t[:], in1=mean_t[:],
                                    op=mybir.AluOpType.mult)
            nc.vector.tensor_tensor(out=t_t[:], in0=gb_t[:, 1:2], in1=t_t[:],
                                    op=mybir.AluOpType.subtract)

            # ---- phase 2: affine + store (feature-major, call-sized tiles) --
            for cg in range(cfg.H * cfg.C):
                e0 = cg * cfg.NCALL
                vb = min(max(cfg.HV - (cg % cfg.C) * cfg.NCALL, 0), cfg.NCALL)
                if vb == 0:
                    continue
                u = outp.tile([128, cfg.NCALL], BF16, tag="u")
                nc.vector.tensor_scalar(
                    out=u[:, :vb], in0=h3res[:, e0:e0 + vb],
                    scalar1=s_t[:], scalar2=t_t[:],
                    op0=mybir.AluOpType.mult, op1=mybir.AluOpType.add)
                nc.sync.dma_start(out=out[:, e0:e0 + vb], in_=u[:, :vb])
    nc.compile()
    return nc


def get_program(cfg):
    key = (cfg.ES, cfg.H, cfg.C, cfg.B, cfg.TN, cfg.E_TOTAL,
           cfg.skip_collective, cfg.skip_gather, cfg.gq, cfg.gbufs, cfg.banded)
    if key not in _PROG_CACHE:
        _PROG_CACHE[key] = build_program(cfg)
    return _PROG_CACHE[key]


def _wrap_idx(flat, cfg):
    """int16 flat idxs [n] -> wrapped [128, n/16] layout (i at [i%16, i//16], x8)."""
    w = flat.reshape(-1, 16).T.astype(np.int16)      # [16, n/16]
    return np.tile(w, (8, 1))                        # [128, n/16]


def host_prep(node_feats, edge_feats, src, dst, W1, b1, W2, b2, W3, b3, gamma, beta,
              cfg=None):
    cfg = cfg or CFG
    nfb = np.asarray(node_feats, np.float32).astype(ml_dtypes.bfloat16)
    efb = np.asarray(edge_feats, np.float32).astype(ml_dtypes.bfloat16)
    src = np.asarray(src)
    dst = np.asarray(dst)
    W1 = np.asarray(W1, np.float32)

    w1e = W1[:ED].astype(ml_dtypes.bfloat16)
    w1m = W1[ED:].astype(ml_dtypes.bfloat16)
    w2b = np.asarray(W2, np.float32).astype(ml_dtypes.bfloat16)
    w3b = np.asarray(W3, np.float32).astype(ml_dtypes.bfloat16)
    bias = np.stack([np.asarray(b1, np.float32),
                     np.asarray(b2, np.float32),
                     np.asarray(b3, np.float32)], axis=1)          # [128, 3]
    gb = np.stack([np.asarray(gamma, np.float32),
                   np.asarray(beta, np.float32)], axis=1)          # [128, 2]

    if cfg.banded:
        order = np.argsort(src, kind="stable")
        global _LAST_ORDER
        _LAST_ORDER = order
        # P = nf @ W1m: node-level message precompute (bf16-rounded inputs)
        P = (nfb.astype(np.float32) @ W1[ED:]).astype(ml_dtypes.bfloat16)
        src = src[order]
        dst = dst[order]
        efb = efb[order]

    in_maps = []
    for c in range(NCORES):
        base = c * cfg.ES
        tabs, sws = [], []
        binfo = []
        for h in range(cfg.H):
            lo = base + h * cfg.HV
            s_h = src[lo:lo + cfg.HV]
            d_h = dst[lo:lo + cfg.HV]
            if cfg.banded:
                u = np.unique(d_h)
            else:
                u = np.unique(np.concatenate([s_h, d_h]))
            assert len(u) <= cfg.TN, f"local table overflow: {len(u)} > {cfg.TN}"
            assert len(u) <= 32768, "int16 index overflow"
            tab = np.zeros((cfg.TN, ND), ml_dtypes.bfloat16)
            tab[:len(u)] = nfb[u]
            tabs.append(tab)
            d16 = np.searchsorted(u, d_h).astype(np.int16)
            pad = cfg.HP - cfg.HV
            if pad:
                d16 = np.concatenate([d16, np.zeros(pad, np.int16)])
            if cfg.banded:
                s_pad = np.concatenate([s_h, np.full(pad, s_h[-1], np.int32)]) if pad                     else s_h.astype(np.int32)
                binfo.append(s_pad)
                for cl in range(cfg.C):
                    sws.append(_wrap_idx(d16[cl * cfg.NCALL:(cl + 1) * cfg.NCALL], cfg))
            else:
                s16 = np.searchsorted(u, s_h).astype(np.int16)
                if pad:
                    s16 = np.concatenate([s16, np.zeros(pad, np.int16)])
                for cl in range(cfg.C):
                    sws.append(_wrap_idx(s16[cl * cfg.NCALL:(cl + 1) * cfg.NCALL], cfg))
                    sws.append(_wrap_idx(d16[cl * cfg.NCALL:(cl + 1) * cfg.NCALL], cfg))
        srcw = np.concatenate(sws, axis=1)
        # edge feats, transposed + per-half padding
        eftc = np.zeros((ED, cfg.EP), ml_dtypes.bfloat16)
        for h in range(cfg.H):
            lo = base + h * cfg.HV
            eftc[:, h * cfg.HP:h * cfg.HP + cfg.HV] = efb[lo:lo + cfg.HV].T
        im = {"srcw": srcw, "eft": eftc,
              "w1e": w1e, "w1m": w1m, "w2": w2b, "w3": w3b,
              "bias": bias, "gb": gb}
        if cfg.banded:
            s_all = np.concatenate(binfo)                 # [EP] padded sorted src
            nbph = cfg.C * cfg.B
            btabc = np.zeros((cfg.NB * 128, ND), ml_dtypes.bfloat16)
            offc = np.zeros((1, cfg.EP), ml_dtypes.bfloat16)
            for b in range(cfg.NB):
                blk = s_all[b * cfg.BLK:(b + 1) * cfg.BLK]
                w0 = int(blk[0])
                assert int(blk.max()) - w0 < 128, f"block {b} span too wide"
                hi = min(w0 + 128, V)
                btabc[b * 128:b * 128 + (hi - w0)] = P[w0:hi]
                offc[0, b * cfg.BLK:(b + 1) * cfg.BLK] = (blk - w0).astype(
                    ml_dtypes.bfloat16)
            im["btab"] = btabc
            im["offr"] = offc
            im["iot"] = np.arange(128, dtype=np.float32).reshape(128, 1)
            im["ones1"] = np.ones((1, 128), ml_dtypes.bfloat16)
        for h in range(cfg.H):
            im[f"tab{h}"] = tabs[h]
        in_maps.append(im)
    return in_maps


_LAST_ORDER = None


def assemble_output(results, cfg=None):
    cfg = cfg or CFG
    out = np.empty((NCORES * cfg.ES, OUT), np.float32)
    for c in range(NCORES):
        oc = np.asarray(results[c]["out"]).astype(np.float32)   # [OUT, EP]
        for h in range(cfg.H):
            lo = c * cfg.ES + h * cfg.HV
            out[lo:lo + cfg.HV] = oc[:, h * cfg.HP:h * cfg.HP + cfg.HV].T
    if cfg.banded:
        unp = np.empty_like(out)
        unp[_LAST_ORDER] = out
        out = unp
    return out


def kernel(**inputs):
    cfg = CFG
    nc = get_program(cfg)
    in_maps = host_prep(**inputs, cfg=cfg)
    res = run_bass_kernel_spmd(nc, in_maps, list(range(NCORES)))
    return assemble_output(res.results, cfg)
